# revision 23
# baseline (speedup 1.0000x reference)
"""Mixtral GQA attention block (B=1, S=2048, HID=4096, NH=32, NKV=8, HD=128),
8-way tensor-parallel over heads on trn2: each core owns 4 query heads + 1 KV
head (one GQA group), computes its partial output projection, host sums the
8 partials.

Device layout notes:
  - All matmul operands are staged transposed (contraction dim on partitions).
    Host pre-packs every tensor partition-major so DMAs are identity copies.
  - The Q/K/V and output projections run as fp8e4 DoubleRow matmuls with a
    3-term hi/lo error compensation: x@w ~= xh@wh + xh@wl + xl@wh. Pieces are
    packed [128, ktile, piece, N] with lhsT pieces (hi, lo) and rhs pieces
    (lo, hi); per ktile pair one DR instr covers both hi@hi products and one
    DR instr per ktile covers both cross products -> 0.75x the f16 PE cost
    at ~1e-3 accuracy. Tensors are pre-scaled into fp8's normal range
    (h x32, weights x1024, attn x16) and the product descale is folded into
    the PSUM eviction ops.
  - Scores are computed transposed (S^T[k,q]) in f16 so the exp'd tiles
    directly serve as lhsT for the P@V matmul; softmax denominator comes from
    an appended ones-column on V; no max-subtraction. Causal masking of the
    diagonal-band blocks is a -60000 bias added INTO the scores PSUM group
    via an identity-lhsT matmul, so exp needs no follow-up mask multiply.
  - RoPE is applied in the transposed orientation; the half-swap crosses
    partitions and runs as two partition-offset SBUF->SBUF DMAs.
  - The attention drain loop is Activation-bound (exp), so independent PE
    work is software-pipelined into it through a pending-work queue:
    chunk 0 streams the q1..q3 projections through attention(0); chunks
    1..3 stream the previous chunk's output projection. Chunk 0's k/v/q0
    projections are 3-way interleaved per ktile pair so the PE tracks the
    cold-start h DMA stream; DMAs are ordered by first use.
"""

import math
import os
import sys

import numpy as np

sys.path.insert(0, "/opt/trn_rl_repo")

import concourse.bass as bass
import concourse.tile as tile
from concourse import bacc
from concourse import mybir

S = 2048
HID = 4096
NH, NKV, HD = 32, 8, 128
NCORES = 8
QH = NH // NCORES      # 4 query heads per core
ND = HID // 128        # 32 contraction chunks
NI = S // 512          # 4 q-chunks of 512
NJ = S // 128          # 16 k-tiles of 128
SCALE = 1.0 / math.sqrt(HD)

F16 = mybir.dt.float16
BF16 = mybir.dt.bfloat16
F32 = mybir.dt.float32
F8 = mybir.dt.float8e4
DR = mybir.MatmulPerfMode.DoubleRow

SH = 32.0        # fp8 pre-scale on h
SW = 1024.0      # fp8 pre-scale on wq/wk/wv/wo
SA = 16.0        # fp8 pre-scale on attention output
DESCALE_QKV = float(1.0 / (SH * SW))
DESCALE_O = float(1.0 / (SA * SW))
MASK_NEG = -60000.0

_CACHE = {}
LAST_RESULTS = None


def _dr_ops(w_ap, h_ap, nd):
    """(lhsT, rhs) DR operand pairs for the 3-term hi/lo product over `nd`
    ktiles, in ktile-streaming order. w pieces are (hi, lo), h (lo, hi)."""
    ops = []
    for a in range(0, nd, 2):
        ops.append((w_ap(a, 0, False), h_ap(a, 1, False)))      # hi@hi pair
        ops.append((w_ap(a, None, True), h_ap(a, None, True)))  # cross a
        ops.append((w_ap(a + 1, None, True), h_ap(a + 1, None, True)))
    return ops


def _build_program():
    nc = bacc.Bacc()

    h4 = nc.declare_dram_parameter("h4", [128, ND, 2, S], F8, isOutput=False)
    wq4 = nc.declare_dram_parameter("wq4", [128, QH, ND, 2, 128], F8,
                                    isOutput=False)
    wk4 = nc.declare_dram_parameter("wk4", [128, ND, 2, 128], F8, isOutput=False)
    wv4 = nc.declare_dram_parameter("wv4", [128, ND, 2, 128], F8, isOutput=False)
    wo4 = nc.declare_dram_parameter("wo4", [128, QH, 2, HID], F8, isOutput=False)
    cosd = nc.declare_dram_parameter("cosd", [128, S], F16, isOutput=False)
    identd = nc.declare_dram_parameter("identd", [128, 128], F16, isOutput=False)
    identb = nc.declare_dram_parameter("identb", [128, 128], BF16, isOutput=False)
    maskd = nc.declare_dram_parameter("maskd", [128, 128], F16, isOutput=False)
    sind = nc.declare_dram_parameter("sind", [128, S], F16, isOutput=False)
    out = nc.declare_dram_parameter("out", [S, HID], F16, isOutput=True)

    with tile.TileContext(nc) as tc:
        with (
            tc.tile_pool(name="consts", bufs=1) as consts,
            tc.tile_pool(name="hpool", bufs=10) as hpool,
            tc.tile_pool(name="ptpool", bufs=5) as ptpool,
            tc.tile_pool(name="rtmp", bufs=3) as rtmp,
            tc.tile_pool(name="small", bufs=8) as small,
            tc.tile_pool(name="orow", bufs=5) as orowp,
            tc.tile_pool(name="pproj", bufs=2, space="PSUM") as pproj,
            tc.tile_pool(name="pwork", bufs=2, space="PSUM") as pwork,
            tc.tile_pool(name="popsum", bufs=4, space="PSUM") as popsum,
        ):
            wq_sb = consts.tile([128, QH, ND, 2, 128], F8)
            cos_sb = consts.tile([128, S], F16)
            sin_sb = consts.tile([128, S], F16)
            wk_sb = consts.tile([128, ND, 2, 128], F8)
            wv_sb = consts.tile([128, ND, 2, 128], F8)
            wo_sb = consts.tile([128, QH, 2, HID], F8)
            ident16 = consts.tile([128, 128], F16)
            identbf = consts.tile([128, 128], BF16)
            maskb = consts.tile([128, 128], F16)

            hts0 = []
            for dq in range(8):
                t_h = hpool.tile([128, 4, 2, 512], F8, tag="ht",
                                 name=f"ht_0_{dq}")
                hts0.append(t_h)

            def ht_dma(tiles, I, dq):
                nc.sync.dma_start(
                    out=tiles[dq],
                    in_=h4[:, dq * 4:(dq + 1) * 4, :, I * 512:(I + 1) * 512],
                )

            def wq_dma(t, half):
                dsl = slice(half * 16, (half + 1) * 16)
                nc.sync.dma_start(out=wq_sb[:, t, dsl, :, :],
                                  in_=wq4[:, t, dsl, :, :])

            # Startup DMA order = first-use order: k/v/q0 weights lead the
            # cold h stream; tables next; q1..q3 heads land during
            # attention(0); wo column blocks land before attention(1).
            nc.sync.dma_start(out=wk_sb[:, 0:4, :, :], in_=wk4[:, 0:4, :, :])
            nc.sync.dma_start(out=hts0[0][:, 0:2, :, :],
                              in_=h4[:, 0:2, :, 0:512])
            nc.sync.dma_start(out=wv_sb[:, 0:4, :, :], in_=wv4[:, 0:4, :, :])
            nc.sync.dma_start(out=wq_sb[:, 0, 0:4, :, :],
                              in_=wq4[:, 0, 0:4, :, :])
            nc.sync.dma_start(out=hts0[0][:, 2:4, :, :],
                              in_=h4[:, 2:4, :, 0:512])
            nc.sync.dma_start(out=wk_sb[:, 4:8, :, :], in_=wk4[:, 4:8, :, :])
            nc.sync.dma_start(out=wv_sb[:, 4:8, :, :], in_=wv4[:, 4:8, :, :])
            nc.sync.dma_start(out=wq_sb[:, 0, 4:8, :, :],
                              in_=wq4[:, 0, 4:8, :, :])
            nc.sync.dma_start(out=wk_sb[:, 8:16, :, :], in_=wk4[:, 8:16, :, :])
            nc.sync.dma_start(out=wv_sb[:, 8:16, :, :], in_=wv4[:, 8:16, :, :])
            nc.sync.dma_start(out=wq_sb[:, 0, 8:16, :, :],
                              in_=wq4[:, 0, 8:16, :, :])
            nc.sync.dma_start(out=wk_sb[:, 16:32, :, :], in_=wk4[:, 16:32, :, :])
            ht_dma(hts0, 0, 1)
            nc.sync.dma_start(out=wv_sb[:, 16:32, :, :], in_=wv4[:, 16:32, :, :])
            ht_dma(hts0, 0, 2)
            wq_dma(0, 1)
            ht_dma(hts0, 0, 3)
            for dq in range(4, 8):
                ht_dma(hts0, 0, dq)
            nc.sync.dma_start(out=cos_sb, in_=cosd[:, :])
            nc.sync.dma_start(out=sin_sb, in_=sind[:, :])
            nc.sync.dma_start(out=identbf, in_=identb[:, :])
            nc.sync.dma_start(out=maskb, in_=maskd[:, :])
            nc.sync.dma_start(out=ident16, in_=identd[:, :])
            for t in range(1, QH):
                wq_dma(t, 0)
                wq_dma(t, 1)

            qT = consts.tile([128, QH, S], F16)    # roped q, transposed
            kT = consts.tile([128, S], F16)        # roped k, transposed
            # V' tiles: per k-tile j, [128 tokens, 128 ch + ones column]
            vA = consts.tile([128, NJ, 132], BF16)
            for j in range(NJ):
                nc.vector.memset(vA[:, j, 128:129], 1.0)
            # attn output pieces for the fp8 O-projection: [hd, head, piece, S]
            attn4 = consts.tile([128, QH, 2, S], F8)

            def w_ap_of(w_tile, t=None):
                if t is None:
                    def w_ap(a, piece, single):
                        if single:
                            return w_tile[:, a, 0:2, :]
                        return w_tile[:, a:a + 2, piece, :]
                else:
                    def w_ap(a, piece, single):
                        if single:
                            return w_tile[:, t, a, 0:2, :]
                        return w_tile[:, t, a:a + 2, piece, :]
                return w_ap

            def h_ap_of(hts):
                def h_ap(a, piece, single):
                    if single:
                        return hts[a // 4][:, a % 4, 0:2, :]
                    return hts[a // 4][:, a % 4:a % 4 + 2, piece, :]
                return h_ap

            def emit_chain(ps, ops, lo, hi_i):
                for i in range(lo, hi_i):
                    nc.tensor.matmul(
                        ps, ops[i][0], ops[i][1],
                        start=(i == 0), stop=(i == len(ops) - 1),
                        perf_mode=DR,
                    )

            def rope_into(ps, dst, nsl):
                """ps: PSUM [128, 512] f32 pre-rope (transposed layout,
                scaled by SH*SW). dst: f16 SBUF slice. The half-swap
                crosses partitions -> two partition-offset DMAs."""
                cpy = rtmp.tile([128, 512], F16, tag="ropecpy")
                nc.scalar.mul(cpy, ps, DESCALE_QKV)
                sw = rtmp.tile([128, 512], F16, tag="ropesw")
                nc.sync.dma_start(out=sw[0:64, :], in_=cpy[64:128, :])
                nc.sync.dma_start(out=sw[64:128, :], in_=cpy[0:64, :])
                tmp2 = rtmp.tile([128, 512], F16, tag="ropecos")
                nc.gpsimd.tensor_mul(tmp2, cpy, cos_sb[:, nsl])
                nc.gpsimd.tensor_mul(sw, sw, sin_sb[:, nsl])
                nc.vector.tensor_add(dst, tmp2, sw)

            def v_stage(v_ps, I):
                vt_sb = small.tile([128, 512], BF16, tag="vt")
                nc.vector.tensor_scalar_mul(vt_sb, v_ps, DESCALE_QKV)
                for jj in range(4):
                    tps = pwork.tile([128, 128], BF16, tag="work")
                    nc.tensor.transpose(
                        tps, vt_sb[:, jj * 128:(jj + 1) * 128], identbf
                    )
                    nc.vector.tensor_copy(vA[:, 4 * I + jj, 0:128], tps)

            # ---- O-projection emission units ------------------------------
            def make_oproj_unit(I, il, qtr, split_dma=False, act_evict=False):
                i_abs = 4 * I + il
                tok = slice(i_abs * 128, (i_abs + 1) * 128)

                def a_ap(a, piece, single):
                    if single:
                        return attn4[:, a, 0:2, tok]
                    return attn4[:, a:a + 2, piece, tok]

                def emit():
                    orow = orowp.tile([128, 1024], F16, tag="orow")
                    for mc in range(2):
                        mq = qtr * 1024 + mc * 512

                        def w_ap(a, piece, single, mq=mq):
                            if single:
                                return wo_sb[:, a, 0:2, mq:mq + 512]
                            return wo_sb[:, a:a + 2, piece, mq:mq + 512]

                        op_ps = pproj.tile([128, 512], F32, tag="proj")
                        ops = _dr_ops(a_ap, w_ap, QH)
                        emit_chain(op_ps, ops, 0, len(ops))
                        osl = orow[:, mc * 512:(mc + 1) * 512]
                        if act_evict and mc == 1:
                            nc.scalar.mul(osl, op_ps, DESCALE_O)
                        else:
                            nc.vector.tensor_scalar_mul(osl, op_ps, DESCALE_O)
                        if split_dma:
                            nc.sync.dma_start(
                                out=out[i_abs * 128:(i_abs + 1) * 128,
                                        mq:mq + 512],
                                in_=osl,
                            )
                    if not split_dma:
                        nc.sync.dma_start(
                            out=out[i_abs * 128:(i_abs + 1) * 128,
                                    qtr * 1024:(qtr + 1) * 1024],
                            in_=orow,
                        )
                return emit

            # ---- chunk 0 cold start: 3-way interleaved k/v/q0 -------------
            nsl0 = slice(0, 512)
            hap0 = h_ap_of(hts0)
            k_ps = pproj.tile([128, 512], F32, tag="proj")
            v_ps = pproj.tile([128, 512], F32, tag="proj")
            q0_ps = popsum.tile([128, 512], F32, tag="opsum", name="q_ps_0_0")
            k_ops = _dr_ops(w_ap_of(wk_sb), hap0, ND)
            v_ops = _dr_ops(w_ap_of(wv_sb), hap0, ND)
            q0_ops = _dr_ops(w_ap_of(wq_sb, 0), hap0, ND)
            for pr in range(ND // 2):
                emit_chain(k_ps, k_ops, 3 * pr, 3 * pr + 3)
                emit_chain(v_ps, v_ops, 3 * pr, 3 * pr + 3)
                emit_chain(q0_ps, q0_ops, 3 * pr, 3 * pr + 3)
            rope_into(k_ps, kT[:, nsl0], nsl0)
            v_stage(v_ps, 0)
            rope_into(q0_ps, qT[:, 0, nsl0], nsl0)

            # chunk-0 pending work: q1..q3 projections streamed through
            # attention(0)'s drains. The chunk-1 h prefetch is emitted right
            # after the LAST q3 group reading each chunk-0 sub-tile (the
            # hpool buffer reuse must come after every reader is emitted),
            # and the wo column blocks trail at the end.
            pending = []
            q_state = {}
            hts_next = []

            def make_q_group(t, pr, hts, nsl):
                def emit():
                    if t not in q_state:
                        q_state[t] = {
                            "ps": pproj.tile([128, 512], F32, tag="proj",
                                             name=f"q_ps_0_{t}"),
                            "ops": _dr_ops(w_ap_of(wq_sb, t), h_ap_of(hts),
                                           ND),
                        }
                    st = q_state[t]
                    emit_chain(st["ps"], st["ops"], 3 * pr, 3 * pr + 3)
                    if pr == ND // 2 - 1:
                        rope_into(st["ps"], qT[:, t, nsl], nsl)
                        del q_state[t]
                return emit

            def make_h1_prefetch(dq):
                def emit():
                    t_h = hpool.tile([128, 4, 2, 512], F8, tag="ht",
                                     name=f"ht_1_{dq}")
                    assert len(hts_next) == dq
                    hts_next.append(t_h)
                    ht_dma(hts_next, 1, dq)
                return emit

            def make_wo_dma(cb):
                def emit():
                    csl = slice(cb * 1024, (cb + 1) * 1024)
                    nc.sync.dma_start(out=wo_sb[:, :, :, csl],
                                      in_=wo4[:, :, :, csl])
                return emit

            # per-head buckets: bucket t is fully emitted during head t's
            # drains, so head t+1's scores (which need qT[t+1]) are never
            # enqueued ahead of the projection work they depend on.
            pending = [[], [], [], []]
            for t in range(1, QH):
                for pr in range(ND // 2):
                    pending[t - 1].append(make_q_group(t, pr, hts0, nsl0))
                    if t == QH - 1 and pr % 2 == 1:
                        pending[t - 1].append(make_h1_prefetch(pr // 2))
            for cb in range(4):
                pending[3].append(make_wo_dma(cb))

            hts_cur = hts_next
            for I in range(NI):
                nsl = slice(I * 512, (I + 1) * 512)
                hts = hts_cur

                if I > 0:
                    # ---- projections (h prefetched; PE-bound) ------------
                    k_ps = pproj.tile([128, 512], F32, tag="proj")
                    emit_chain(k_ps, _dr_ops(w_ap_of(wk_sb), h_ap_of(hts),
                                             ND), 0, 3 * (ND // 2))
                    rope_into(k_ps, kT[:, nsl], nsl)

                    v_ps = pproj.tile([128, 512], F32, tag="proj")
                    emit_chain(v_ps, _dr_ops(w_ap_of(wv_sb), h_ap_of(hts),
                                             ND), 0, 3 * (ND // 2))
                    v_stage(v_ps, I)

                    for t in range(QH):
                        q_ps = pproj.tile([128, 512], F32, tag="proj")
                        emit_chain(q_ps, _dr_ops(w_ap_of(wq_sb, t),
                                                 h_ap_of(hts), ND),
                                   0, 3 * (ND // 2))
                        rope_into(q_ps, qT[:, t, nsl], nsl)

                    # prefetch next pass's h slice
                    if I + 1 < NI:
                        hts_cur = []
                        for dq in range(8):
                            t_h = hpool.tile([128, 4, 2, 512], F8, tag="ht",
                                             name=f"ht_{I + 1}_{dq}")
                            hts_cur.append(t_h)
                            ht_dma(hts_cur, I + 1, dq)

                # ---- attention for q-chunk I, with pending PE work
                # interleaved into the drain loop ----------------------
                njt = 4 * I + 4   # k-tiles in causal range of this chunk
                state = {}

                def start_head(t, I=I):
                    o_ps = [
                        popsum.tile([128, 132], F32, tag="opsum",
                                    name=f"o_ps_{I}_{t}_{il}")
                        for il in range(4)
                    ]
                    return {"o_ps": o_ps}

                def finalize_il(t, il, I=I):
                    o_ps = state[t]["o_ps"]
                    i_abs = 4 * I + il
                    recip = small.tile([128, 1], F32, tag="recip")
                    nc.vector.reciprocal(recip, o_ps[il][:, 128:129])
                    osc = small.tile([128, 128], F16, tag="osc")
                    nc.vector.tensor_scalar_mul(
                        osc, o_ps[il][:, 0:128], recip
                    )
                    tps = popsum.tile([128, 132], F16, tag="opsum",
                                      name=f"tps_{I}_{t}_{il}")
                    nc.tensor.transpose(tps[:, 0:128], osc, ident16)
                    tok = slice(i_abs * 128, (i_abs + 1) * 128)
                    nc.scalar.mul(attn4[:, t, 0, tok], tps[:, 0:128], SA)
                    nc.vector.scalar_tensor_tensor(
                        attn4[:, t, 1, tok], tps[:, 0:128], SA,
                        attn4[:, t, 0, tok],
                        mybir.AluOpType.mult, mybir.AluOpType.subtract,
                    )

                def drain(t, jprev, s_ps, I=I):
                    m = jprev - 4 * I
                    q_off = 128 * m if m > 0 else 0
                    pt = ptpool.tile([128, 512], BF16, tag="pt")
                    nc.scalar.activation(
                        pt[:, q_off:512], s_ps[:, q_off:512],
                        mybir.ActivationFunctionType.Exp,
                        scale=SCALE,
                    )
                    o_ps = state[t]["o_ps"]
                    for il in range(4):
                        i_abs = 4 * I + il
                        if jprev <= i_abs:
                            nc.tensor.matmul(
                                o_ps[il][:, 0:129],
                                pt[:, il * 128:(il + 1) * 128],
                                vA[:, jprev, 0:129],
                                start=(jprev == 0),
                                stop=(jprev == i_abs),
                            )
                            if jprev == i_abs:
                                finalize_il(t, il)

                prev = None
                for t in range(QH):
                    state[t] = start_head(t)
                    bucket = pending[t]
                    bucket_total = len(bucket)
                    bucket_done = 0
                    for j in range(njt):
                        mj = j - 4 * I
                        q_off = 128 * mj if mj > 0 else 0
                        s_ps = pwork.tile([128, 512], F32, tag="work")
                        diag = mj >= 0
                        nc.tensor.matmul(
                            s_ps[:, q_off:512],
                            kT[:, j * 128:(j + 1) * 128],
                            qT[:, t, I * 512 + q_off:(I + 1) * 512],
                            start=True, stop=not diag,
                        )
                        if diag:
                            # -60000 causal bias onto the diagonal block
                            nc.tensor.matmul(
                                s_ps[:, q_off:q_off + 128],
                                ident16, maskb,
                                start=False, stop=True,
                                skip_group_check=True,
                            )
                        if prev is not None:
                            tp_, jp_, sp_ = prev
                            drain(tp_, jp_, sp_)
                            target = (j + 1) * bucket_total // njt
                            while bucket and bucket_done < target:
                                bucket.pop(0)()
                                bucket_done += 1
                        elif bucket:
                            # prime the PE while the first exp runs
                            for _ in range(min(2, len(bucket))):
                                bucket.pop(0)()
                                bucket_done += 1
                        prev = (t, j, s_ps)
                    for fn in bucket:
                        fn()
                tp_, jp_, sp_ = prev
                drain(tp_, jp_, sp_)
                last = I == NI - 1
                units = [make_oproj_unit(I, il, qtr, split_dma=last,
                                          act_evict=last)
                         for il in range(4) for qtr in range(4)]
                pending = [units[0:4], units[4:8], units[8:12], units[12:16]]

            for bucket in pending:
                for fn in bucket:
                    fn()
    nc.finalize()
    return nc


def _hilo(x, scale):
    """Split x*scale into fp8e4 (hi, lo) pieces."""
    import ml_dtypes

    xs = (x * scale).astype(np.float32)
    hi = xs.astype(ml_dtypes.float8_e4m3)
    lo = (xs - hi.astype(np.float32)).astype(ml_dtypes.float8_e4m3)
    return hi, lo


def _pack4(xT, scale, nd, ncols, lhs):
    """Pack xT [nd*128, ncols] into [128, nd, 2, ncols] fp8 pieces.
    lhs=True -> pieces (hi, lo); False -> (lo, hi)."""
    import ml_dtypes

    hi, lo = _hilo(xT, scale)
    outp = np.empty((128, nd, 2, ncols), dtype=ml_dtypes.float8_e4m3)
    h3 = hi.reshape(nd, 128, ncols)
    l3 = lo.reshape(nd, 128, ncols)
    if lhs:
        outp[:, :, 0, :] = h3.transpose(1, 0, 2)
        outp[:, :, 1, :] = l3.transpose(1, 0, 2)
    else:
        outp[:, :, 0, :] = l3.transpose(1, 0, 2)
        outp[:, :, 1, :] = h3.transpose(1, 0, 2)
    return outp


def _pack_inputs(h, position_ids, wq, wk, wv, wo):
    """Host-side shard + transpose + fp8 hi/lo split. Per-core input maps."""
    import ml_dtypes

    h4 = _pack4(np.ascontiguousarray(h.T), SH, ND, S, lhs=False)

    # RoPE tables in transposed orientation, halves duplicated / sign-folded.
    inv = 1.0 / (1e6 ** (np.arange(0, HD, 2, dtype=np.float64) / HD))
    fr = position_ids.astype(np.float64)[None, :] * inv[:, None]   # [64, S]
    cosT = np.cos(fr).astype(np.float16)
    sinT = np.sin(fr).astype(np.float16)
    cosd = np.concatenate([cosT, cosT], axis=0)                    # [128, S]
    sind = np.concatenate([-sinT, sinT], axis=0)
    iden16 = np.eye(128, dtype=np.float16)
    idenbf = np.eye(128).astype(ml_dtypes.bfloat16)
    p_i = np.arange(128)[:, None]
    f_i = np.arange(128)[None, :]
    maskd = np.where(f_i - p_i >= 0, 0.0, MASK_NEG).astype(np.float16)

    in_maps = []
    for c in range(NCORES):
        wq_c = wq[c * 512:(c + 1) * 512, :]          # [512, HID]
        wk_c = wk[c * 128:(c + 1) * 128, :]
        wv_c = wv[c * 128:(c + 1) * 128, :]
        wo_c = wo[:, c * 512:(c + 1) * 512]          # [HID, 512]
        woT = np.ascontiguousarray(wo_c.T)           # [512, HID]
        wqT = np.ascontiguousarray(wq_c.T)           # [HID, 512]
        wq4 = np.stack(
            [_pack4(np.ascontiguousarray(wqT[:, t * 128:(t + 1) * 128]),
                    SW, ND, 128, lhs=True) for t in range(QH)],
            axis=1,
        )                                            # [128, QH, ND, 2, 128]
        in_maps.append({
            "h4": h4,
            "wq4": wq4,
            "wk4": _pack4(np.ascontiguousarray(wk_c.T), SW, ND, 128, lhs=True),
            "wv4": _pack4(np.ascontiguousarray(wv_c.T), SW, ND, 128, lhs=True),
            "wo4": _pack4(woT, SW, QH, HID, lhs=False),
            "cosd": cosd,
            "sind": sind,
            "identd": iden16,
            "identb": idenbf,
            "maskd": maskd,
        })
    return in_maps


def kernel(h, position_ids, wq, wk, wv, wo):
    global LAST_RESULTS
    from concourse.bass_utils import run_bass_kernel_spmd

    if "nc" not in _CACHE:
        _CACHE["nc"] = _build_program()
    nc = _CACHE["nc"]

    in_maps = _pack_inputs(
        np.asarray(h, dtype=np.float32),
        np.asarray(position_ids),
        np.asarray(wq, dtype=np.float32),
        np.asarray(wk, dtype=np.float32),
        np.asarray(wv, dtype=np.float32),
        np.asarray(wo, dtype=np.float32),
    )

    trace = bool(int(os.environ.get("KERNEL_TRACE", "0")))
    res = run_bass_kernel_spmd(
        nc, in_maps, core_ids=list(range(NCORES)), trace=trace
    )
    LAST_RESULTS = res

    acc = np.zeros((S, HID), dtype=np.float32)
    for r in res.results:
        acc += r["out"].astype(np.float32)
    return acc


# revision 24
# speedup vs baseline: 1.0183x; 1.0183x over previous
"""Mixtral GQA attention block (B=1, S=2048, HID=4096, NH=32, NKV=8, HD=128),
8-way tensor-parallel over heads on trn2: each core owns 4 query heads + 1 KV
head (one GQA group), computes its partial output projection, host sums the
8 partials.

Device layout notes:
  - All matmul operands are staged transposed (contraction dim on partitions).
    Host pre-packs every tensor partition-major so DMAs are identity copies.
  - The Q/K/V and output projections run as fp8e4 DoubleRow matmuls with a
    3-term hi/lo error compensation: x@w ~= xh@wh + xh@wl + xl@wh. Pieces are
    packed [128, ktile, piece, N] with lhsT pieces (hi, lo) and rhs pieces
    (lo, hi); per ktile pair one DR instr covers both hi@hi products and one
    DR instr per ktile covers both cross products -> 0.75x the f16 PE cost
    at ~1e-3 accuracy. Tensors are pre-scaled into fp8's normal range
    (h x32, weights x1024, attn x16) and the product descale is folded into
    the PSUM eviction ops.
  - Scores are computed transposed (S^T[k,q]) in f16 so the exp'd tiles
    directly serve as lhsT for the P@V matmul; softmax denominator comes from
    an appended ones-column on V; no max-subtraction. Causal masking of the
    diagonal-band blocks is a -60000 bias added INTO the scores PSUM group
    via an identity-lhsT matmul, so exp needs no follow-up mask multiply.
  - RoPE is applied in the transposed orientation; the half-swap crosses
    partitions and runs as two partition-offset SBUF->SBUF DMAs.
  - The attention drain loop is Activation-bound (exp), so independent PE
    work is software-pipelined into it through a pending-work queue:
    chunk 0 streams the q1..q3 projections through attention(0); chunks
    1..3 stream the previous chunk's output projection. Chunk 0's k/v/q0
    projections are 3-way interleaved per ktile pair so the PE tracks the
    cold-start h DMA stream; DMAs are ordered by first use.
"""

import math
import os
import sys

import numpy as np

sys.path.insert(0, "/opt/trn_rl_repo")

import concourse.bass as bass
import concourse.tile as tile
from concourse import bacc
from concourse import mybir

S = 2048
HID = 4096
NH, NKV, HD = 32, 8, 128
NCORES = 8
QH = NH // NCORES      # 4 query heads per core
ND = HID // 128        # 32 contraction chunks
NI = S // 512          # 4 q-chunks of 512
NJ = S // 128          # 16 k-tiles of 128
SCALE = 1.0 / math.sqrt(HD)

F16 = mybir.dt.float16
BF16 = mybir.dt.bfloat16
F32 = mybir.dt.float32
F8 = mybir.dt.float8e4
DR = mybir.MatmulPerfMode.DoubleRow

SH = 32.0        # fp8 pre-scale on h
SW = 1024.0      # fp8 pre-scale on wq/wk/wv/wo
SA = 16.0        # fp8 pre-scale on attention output
DESCALE_QKV = float(1.0 / (SH * SW))
DESCALE_O = float(1.0 / (SA * SW))
MASK_NEG = -60000.0

_CACHE = {}
LAST_RESULTS = None


def _dr_ops(w_ap, h_ap, nd):
    """(lhsT, rhs) DR operand pairs for the 3-term hi/lo product over `nd`
    ktiles, in ktile-streaming order. w pieces are (hi, lo), h (lo, hi)."""
    ops = []
    for a in range(0, nd, 2):
        ops.append((w_ap(a, 0, False), h_ap(a, 1, False)))      # hi@hi pair
        ops.append((w_ap(a, None, True), h_ap(a, None, True)))  # cross a
        ops.append((w_ap(a + 1, None, True), h_ap(a + 1, None, True)))
    return ops


def _build_program():
    nc = bacc.Bacc()

    h4 = nc.declare_dram_parameter("h4", [128, ND, 2, S], F8, isOutput=False)
    wq4 = nc.declare_dram_parameter("wq4", [128, QH, ND, 2, 128], F8,
                                    isOutput=False)
    wk4 = nc.declare_dram_parameter("wk4", [128, ND, 2, 128], F8, isOutput=False)
    wv4 = nc.declare_dram_parameter("wv4", [128, ND, 2, 128], F8, isOutput=False)
    wo4 = nc.declare_dram_parameter("wo4", [128, QH, 2, HID], F8, isOutput=False)
    cosd = nc.declare_dram_parameter("cosd", [128, S], F16, isOutput=False)
    identd = nc.declare_dram_parameter("identd", [128, 128], F16, isOutput=False)
    identb = nc.declare_dram_parameter("identb", [128, 128], BF16, isOutput=False)
    maskd = nc.declare_dram_parameter("maskd", [128, 128], F16, isOutput=False)
    sind = nc.declare_dram_parameter("sind", [128, S], F16, isOutput=False)
    out = nc.declare_dram_parameter("out", [S, HID], F16, isOutput=True)

    with tile.TileContext(nc) as tc:
        with (
            tc.tile_pool(name="consts", bufs=1) as consts,
            tc.tile_pool(name="hpool", bufs=10) as hpool,
            tc.tile_pool(name="ptpool", bufs=5) as ptpool,
            tc.tile_pool(name="rtmp", bufs=3) as rtmp,
            tc.tile_pool(name="small", bufs=8) as small,
            tc.tile_pool(name="orow", bufs=5) as orowp,
            tc.tile_pool(name="pproj", bufs=2, space="PSUM") as pproj,
            tc.tile_pool(name="pwork", bufs=2, space="PSUM") as pwork,
            tc.tile_pool(name="popsum", bufs=4, space="PSUM") as popsum,
        ):
            wq_sb = consts.tile([128, QH, ND, 2, 128], F8)
            cos_sb = consts.tile([128, S], F16)
            sin_sb = consts.tile([128, S], F16)
            wk_sb = consts.tile([128, ND, 2, 128], F8)
            wv_sb = consts.tile([128, ND, 2, 128], F8)
            wo_sb = consts.tile([128, QH, 2, HID], F8)
            ident16 = consts.tile([128, 128], F16)
            identbf = consts.tile([128, 128], BF16)
            maskb = consts.tile([128, 128], F16)

            hts0 = []
            for dq in range(8):
                t_h = hpool.tile([128, 4, 2, 512], F8, tag="ht",
                                 name=f"ht_0_{dq}")
                hts0.append(t_h)

            def ht_dma(tiles, I, dq):
                nc.sync.dma_start(
                    out=tiles[dq],
                    in_=h4[:, dq * 4:(dq + 1) * 4, :, I * 512:(I + 1) * 512],
                )

            def wq_dma(t, half):
                dsl = slice(half * 16, (half + 1) * 16)
                nc.sync.dma_start(out=wq_sb[:, t, dsl, :, :],
                                  in_=wq4[:, t, dsl, :, :])

            # Startup DMA order = first-use order: k/v/q0 weights lead the
            # cold h stream; tables next; q1..q3 heads land during
            # attention(0); wo column blocks land before attention(1).
            nc.sync.dma_start(out=wk_sb[:, 0:8, :, :], in_=wk4[:, 0:8, :, :])
            nc.sync.dma_start(out=hts0[0][:, 0:2, :, :],
                              in_=h4[:, 0:2, :, 0:512])
            nc.sync.dma_start(out=wv_sb[:, 0:8, :, :], in_=wv4[:, 0:8, :, :])
            nc.sync.dma_start(out=hts0[0][:, 2:4, :, :],
                              in_=h4[:, 2:4, :, 0:512])
            nc.sync.dma_start(out=wq_sb[:, 0, 0:8, :, :],
                              in_=wq4[:, 0, 0:8, :, :])
            nc.sync.dma_start(out=wk_sb[:, 8:16, :, :], in_=wk4[:, 8:16, :, :])
            nc.sync.dma_start(out=wv_sb[:, 8:16, :, :], in_=wv4[:, 8:16, :, :])
            nc.sync.dma_start(out=wq_sb[:, 0, 8:16, :, :],
                              in_=wq4[:, 0, 8:16, :, :])
            nc.sync.dma_start(out=wk_sb[:, 16:32, :, :], in_=wk4[:, 16:32, :, :])
            ht_dma(hts0, 0, 1)
            nc.sync.dma_start(out=wv_sb[:, 16:32, :, :], in_=wv4[:, 16:32, :, :])
            ht_dma(hts0, 0, 2)
            wq_dma(0, 1)
            ht_dma(hts0, 0, 3)
            for dq in range(4, 8):
                ht_dma(hts0, 0, dq)
            nc.sync.dma_start(out=cos_sb, in_=cosd[:, :])
            nc.sync.dma_start(out=sin_sb, in_=sind[:, :])
            nc.sync.dma_start(out=identbf, in_=identb[:, :])
            nc.sync.dma_start(out=maskb, in_=maskd[:, :])
            nc.sync.dma_start(out=ident16, in_=identd[:, :])
            for t in range(1, QH):
                wq_dma(t, 0)
                wq_dma(t, 1)

            qT = consts.tile([128, QH, S], F16)    # roped q, transposed
            kT = consts.tile([128, S], F16)        # roped k, transposed
            # V' tiles: per k-tile j, [128 tokens, 128 ch + ones column]
            vA = consts.tile([128, NJ, 132], BF16)
            for j in range(NJ):
                nc.vector.memset(vA[:, j, 128:129], 1.0)
            # attn output pieces for the fp8 O-projection: [hd, head, piece, S]
            attn4 = consts.tile([128, QH, 2, S], F8)

            def w_ap_of(w_tile, t=None):
                if t is None:
                    def w_ap(a, piece, single):
                        if single:
                            return w_tile[:, a, 0:2, :]
                        return w_tile[:, a:a + 2, piece, :]
                else:
                    def w_ap(a, piece, single):
                        if single:
                            return w_tile[:, t, a, 0:2, :]
                        return w_tile[:, t, a:a + 2, piece, :]
                return w_ap

            def h_ap_of(hts):
                def h_ap(a, piece, single):
                    if single:
                        return hts[a // 4][:, a % 4, 0:2, :]
                    return hts[a // 4][:, a % 4:a % 4 + 2, piece, :]
                return h_ap

            def emit_chain(ps, ops, lo, hi_i):
                for i in range(lo, hi_i):
                    nc.tensor.matmul(
                        ps, ops[i][0], ops[i][1],
                        start=(i == 0), stop=(i == len(ops) - 1),
                        perf_mode=DR,
                    )

            def rope_into(ps, dst, nsl):
                """ps: PSUM [128, 512] f32 pre-rope (transposed layout,
                scaled by SH*SW). dst: f16 SBUF slice. The half-swap
                crosses partitions -> two partition-offset DMAs."""
                cpy = rtmp.tile([128, 512], F16, tag="ropecpy")
                nc.scalar.mul(cpy, ps, DESCALE_QKV)
                sw = rtmp.tile([128, 512], F16, tag="ropesw")
                nc.sync.dma_start(out=sw[0:64, :], in_=cpy[64:128, :])
                nc.sync.dma_start(out=sw[64:128, :], in_=cpy[0:64, :])
                tmp2 = rtmp.tile([128, 512], F16, tag="ropecos")
                nc.gpsimd.tensor_mul(tmp2, cpy, cos_sb[:, nsl])
                nc.gpsimd.tensor_mul(sw, sw, sin_sb[:, nsl])
                nc.vector.tensor_add(dst, tmp2, sw)

            def v_stage(v_ps, I):
                vt_sb = small.tile([128, 512], BF16, tag="vt")
                nc.vector.tensor_scalar_mul(vt_sb, v_ps, DESCALE_QKV)
                for jj in range(4):
                    tps = pwork.tile([128, 128], BF16, tag="work")
                    nc.tensor.transpose(
                        tps, vt_sb[:, jj * 128:(jj + 1) * 128], identbf
                    )
                    nc.vector.tensor_copy(vA[:, 4 * I + jj, 0:128], tps)

            # ---- O-projection emission units ------------------------------
            def make_oproj_unit(I, il, qtr, split_dma=False, act_evict=False):
                i_abs = 4 * I + il
                tok = slice(i_abs * 128, (i_abs + 1) * 128)

                def a_ap(a, piece, single):
                    if single:
                        return attn4[:, a, 0:2, tok]
                    return attn4[:, a:a + 2, piece, tok]

                def emit():
                    orow = orowp.tile([128, 1024], F16, tag="orow")
                    for mc in range(2):
                        mq = qtr * 1024 + mc * 512

                        def w_ap(a, piece, single, mq=mq):
                            if single:
                                return wo_sb[:, a, 0:2, mq:mq + 512]
                            return wo_sb[:, a:a + 2, piece, mq:mq + 512]

                        op_ps = pproj.tile([128, 512], F32, tag="proj")
                        ops = _dr_ops(a_ap, w_ap, QH)
                        emit_chain(op_ps, ops, 0, len(ops))
                        osl = orow[:, mc * 512:(mc + 1) * 512]
                        if act_evict and mc == 1:
                            nc.scalar.mul(osl, op_ps, DESCALE_O)
                        else:
                            nc.vector.tensor_scalar_mul(osl, op_ps, DESCALE_O)
                        if split_dma:
                            nc.sync.dma_start(
                                out=out[i_abs * 128:(i_abs + 1) * 128,
                                        mq:mq + 512],
                                in_=osl,
                            )
                    if not split_dma:
                        nc.sync.dma_start(
                            out=out[i_abs * 128:(i_abs + 1) * 128,
                                    qtr * 1024:(qtr + 1) * 1024],
                            in_=orow,
                        )
                return emit

            # ---- chunk 0 cold start: 3-way interleaved k/v/q0 -------------
            nsl0 = slice(0, 512)
            hap0 = h_ap_of(hts0)
            k_ps = pproj.tile([128, 512], F32, tag="proj")
            v_ps = pproj.tile([128, 512], F32, tag="proj")
            q0_ps = popsum.tile([128, 512], F32, tag="opsum", name="q_ps_0_0")
            k_ops = _dr_ops(w_ap_of(wk_sb), hap0, ND)
            v_ops = _dr_ops(w_ap_of(wv_sb), hap0, ND)
            q0_ops = _dr_ops(w_ap_of(wq_sb, 0), hap0, ND)
            for pr in range(ND // 2):
                emit_chain(k_ps, k_ops, 3 * pr, 3 * pr + 3)
                emit_chain(v_ps, v_ops, 3 * pr, 3 * pr + 3)
                emit_chain(q0_ps, q0_ops, 3 * pr, 3 * pr + 3)
            rope_into(k_ps, kT[:, nsl0], nsl0)
            v_stage(v_ps, 0)
            rope_into(q0_ps, qT[:, 0, nsl0], nsl0)

            # chunk-0 pending work: q1..q3 projections streamed through
            # attention(0)'s drains. The chunk-1 h prefetch is emitted right
            # after the LAST q3 group reading each chunk-0 sub-tile (the
            # hpool buffer reuse must come after every reader is emitted),
            # and the wo column blocks trail at the end.
            pending = []
            q_state = {}
            hts_next = []

            def make_q_group(t, pr, hts, nsl):
                def emit():
                    if t not in q_state:
                        q_state[t] = {
                            "ps": pproj.tile([128, 512], F32, tag="proj",
                                             name=f"q_ps_0_{t}"),
                            "ops": _dr_ops(w_ap_of(wq_sb, t), h_ap_of(hts),
                                           ND),
                        }
                    st = q_state[t]
                    emit_chain(st["ps"], st["ops"], 3 * pr, 3 * pr + 3)
                    if pr == ND // 2 - 1:
                        rope_into(st["ps"], qT[:, t, nsl], nsl)
                        del q_state[t]
                return emit

            def make_h1_prefetch(dq):
                def emit():
                    t_h = hpool.tile([128, 4, 2, 512], F8, tag="ht",
                                     name=f"ht_1_{dq}")
                    assert len(hts_next) == dq
                    hts_next.append(t_h)
                    ht_dma(hts_next, 1, dq)
                return emit

            def make_wo_dma(cb):
                def emit():
                    csl = slice(cb * 1024, (cb + 1) * 1024)
                    nc.sync.dma_start(out=wo_sb[:, :, :, csl],
                                      in_=wo4[:, :, :, csl])
                return emit

            # per-head buckets: bucket t is fully emitted during head t's
            # drains, so head t+1's scores (which need qT[t+1]) are never
            # enqueued ahead of the projection work they depend on.
            pending = [[], [], [], []]
            for t in range(1, QH):
                for pr in range(ND // 2):
                    pending[t - 1].append(make_q_group(t, pr, hts0, nsl0))
                    if t == QH - 1 and pr % 2 == 1:
                        pending[t - 1].append(make_h1_prefetch(pr // 2))
            for cb in range(4):
                pending[3].append(make_wo_dma(cb))

            hts_cur = hts_next
            for I in range(NI):
                nsl = slice(I * 512, (I + 1) * 512)
                hts = hts_cur

                if I > 0:
                    # ---- projections (h prefetched; PE-bound) ------------
                    k_ps = pproj.tile([128, 512], F32, tag="proj")
                    emit_chain(k_ps, _dr_ops(w_ap_of(wk_sb), h_ap_of(hts),
                                             ND), 0, 3 * (ND // 2))
                    rope_into(k_ps, kT[:, nsl], nsl)

                    v_ps = pproj.tile([128, 512], F32, tag="proj")
                    emit_chain(v_ps, _dr_ops(w_ap_of(wv_sb), h_ap_of(hts),
                                             ND), 0, 3 * (ND // 2))
                    v_stage(v_ps, I)

                    for t in range(QH):
                        q_ps = pproj.tile([128, 512], F32, tag="proj")
                        emit_chain(q_ps, _dr_ops(w_ap_of(wq_sb, t),
                                                 h_ap_of(hts), ND),
                                   0, 3 * (ND // 2))
                        rope_into(q_ps, qT[:, t, nsl], nsl)

                    # prefetch next pass's h slice
                    if I + 1 < NI:
                        hts_cur = []
                        for dq in range(8):
                            t_h = hpool.tile([128, 4, 2, 512], F8, tag="ht",
                                             name=f"ht_{I + 1}_{dq}")
                            hts_cur.append(t_h)
                            ht_dma(hts_cur, I + 1, dq)

                # ---- attention for q-chunk I, with pending PE work
                # interleaved into the drain loop ----------------------
                njt = 4 * I + 4   # k-tiles in causal range of this chunk
                state = {}

                def start_head(t, I=I):
                    o_ps = [
                        popsum.tile([128, 132], F32, tag="opsum",
                                    name=f"o_ps_{I}_{t}_{il}")
                        for il in range(4)
                    ]
                    return {"o_ps": o_ps}

                def finalize_il(t, il, I=I):
                    o_ps = state[t]["o_ps"]
                    i_abs = 4 * I + il
                    recip = small.tile([128, 1], F32, tag="recip")
                    nc.vector.reciprocal(recip, o_ps[il][:, 128:129])
                    osc = small.tile([128, 128], F16, tag="osc")
                    nc.vector.tensor_scalar_mul(
                        osc, o_ps[il][:, 0:128], recip
                    )
                    tps = popsum.tile([128, 132], F16, tag="opsum",
                                      name=f"tps_{I}_{t}_{il}")
                    nc.tensor.transpose(tps[:, 0:128], osc, ident16)
                    tok = slice(i_abs * 128, (i_abs + 1) * 128)
                    nc.scalar.mul(attn4[:, t, 0, tok], tps[:, 0:128], SA)
                    nc.vector.scalar_tensor_tensor(
                        attn4[:, t, 1, tok], tps[:, 0:128], SA,
                        attn4[:, t, 0, tok],
                        mybir.AluOpType.mult, mybir.AluOpType.subtract,
                    )

                def drain(t, jprev, s_ps, I=I):
                    m = jprev - 4 * I
                    q_off = 128 * m if m > 0 else 0
                    pt = ptpool.tile([128, 512], BF16, tag="pt")
                    nc.scalar.activation(
                        pt[:, q_off:512], s_ps[:, q_off:512],
                        mybir.ActivationFunctionType.Exp,
                        scale=SCALE,
                    )
                    o_ps = state[t]["o_ps"]
                    for il in range(4):
                        i_abs = 4 * I + il
                        if jprev <= i_abs:
                            nc.tensor.matmul(
                                o_ps[il][:, 0:129],
                                pt[:, il * 128:(il + 1) * 128],
                                vA[:, jprev, 0:129],
                                start=(jprev == 0),
                                stop=(jprev == i_abs),
                            )
                            if jprev == i_abs:
                                finalize_il(t, il)

                prev = None
                for t in range(QH):
                    state[t] = start_head(t)
                    bucket = pending[t]
                    bucket_total = len(bucket)
                    bucket_done = 0
                    for j in range(njt):
                        mj = j - 4 * I
                        q_off = 128 * mj if mj > 0 else 0
                        s_ps = pwork.tile([128, 512], F32, tag="work")
                        diag = mj >= 0
                        nc.tensor.matmul(
                            s_ps[:, q_off:512],
                            kT[:, j * 128:(j + 1) * 128],
                            qT[:, t, I * 512 + q_off:(I + 1) * 512],
                            start=True, stop=not diag,
                        )
                        if diag:
                            # -60000 causal bias onto the diagonal block
                            nc.tensor.matmul(
                                s_ps[:, q_off:q_off + 128],
                                ident16, maskb,
                                start=False, stop=True,
                                skip_group_check=True,
                            )
                        if prev is not None:
                            tp_, jp_, sp_ = prev
                            drain(tp_, jp_, sp_)
                            target = (j + 1) * bucket_total // njt
                            while bucket and bucket_done < target:
                                bucket.pop(0)()
                                bucket_done += 1
                        elif bucket:
                            # prime the PE while the first exp runs
                            bucket.pop(0)()
                            bucket_done += 1
                        prev = (t, j, s_ps)
                    for fn in bucket:
                        fn()
                tp_, jp_, sp_ = prev
                drain(tp_, jp_, sp_)
                last = I == NI - 1
                units = [make_oproj_unit(I, il, qtr, split_dma=last,
                                          act_evict=last)
                         for il in range(4) for qtr in range(4)]
                pending = [units[0:4], units[4:8], units[8:12], units[12:16]]

            for bucket in pending:
                for fn in bucket:
                    fn()
    nc.finalize()
    return nc


def _hilo(x, scale):
    """Split x*scale into fp8e4 (hi, lo) pieces."""
    import ml_dtypes

    xs = (x * scale).astype(np.float32)
    hi = xs.astype(ml_dtypes.float8_e4m3)
    lo = (xs - hi.astype(np.float32)).astype(ml_dtypes.float8_e4m3)
    return hi, lo


def _pack4(xT, scale, nd, ncols, lhs):
    """Pack xT [nd*128, ncols] into [128, nd, 2, ncols] fp8 pieces.
    lhs=True -> pieces (hi, lo); False -> (lo, hi)."""
    import ml_dtypes

    hi, lo = _hilo(xT, scale)
    outp = np.empty((128, nd, 2, ncols), dtype=ml_dtypes.float8_e4m3)
    h3 = hi.reshape(nd, 128, ncols)
    l3 = lo.reshape(nd, 128, ncols)
    if lhs:
        outp[:, :, 0, :] = h3.transpose(1, 0, 2)
        outp[:, :, 1, :] = l3.transpose(1, 0, 2)
    else:
        outp[:, :, 0, :] = l3.transpose(1, 0, 2)
        outp[:, :, 1, :] = h3.transpose(1, 0, 2)
    return outp


def _pack_inputs(h, position_ids, wq, wk, wv, wo):
    """Host-side shard + transpose + fp8 hi/lo split. Per-core input maps."""
    import ml_dtypes

    h4 = _pack4(np.ascontiguousarray(h.T), SH, ND, S, lhs=False)

    # RoPE tables in transposed orientation, halves duplicated / sign-folded.
    inv = 1.0 / (1e6 ** (np.arange(0, HD, 2, dtype=np.float64) / HD))
    fr = position_ids.astype(np.float64)[None, :] * inv[:, None]   # [64, S]
    cosT = np.cos(fr).astype(np.float16)
    sinT = np.sin(fr).astype(np.float16)
    cosd = np.concatenate([cosT, cosT], axis=0)                    # [128, S]
    sind = np.concatenate([-sinT, sinT], axis=0)
    iden16 = np.eye(128, dtype=np.float16)
    idenbf = np.eye(128).astype(ml_dtypes.bfloat16)
    p_i = np.arange(128)[:, None]
    f_i = np.arange(128)[None, :]
    maskd = np.where(f_i - p_i >= 0, 0.0, MASK_NEG).astype(np.float16)

    in_maps = []
    for c in range(NCORES):
        wq_c = wq[c * 512:(c + 1) * 512, :]          # [512, HID]
        wk_c = wk[c * 128:(c + 1) * 128, :]
        wv_c = wv[c * 128:(c + 1) * 128, :]
        wo_c = wo[:, c * 512:(c + 1) * 512]          # [HID, 512]
        woT = np.ascontiguousarray(wo_c.T)           # [512, HID]
        wqT = np.ascontiguousarray(wq_c.T)           # [HID, 512]
        wq4 = np.stack(
            [_pack4(np.ascontiguousarray(wqT[:, t * 128:(t + 1) * 128]),
                    SW, ND, 128, lhs=True) for t in range(QH)],
            axis=1,
        )                                            # [128, QH, ND, 2, 128]
        in_maps.append({
            "h4": h4,
            "wq4": wq4,
            "wk4": _pack4(np.ascontiguousarray(wk_c.T), SW, ND, 128, lhs=True),
            "wv4": _pack4(np.ascontiguousarray(wv_c.T), SW, ND, 128, lhs=True),
            "wo4": _pack4(woT, SW, QH, HID, lhs=False),
            "cosd": cosd,
            "sind": sind,
            "identd": iden16,
            "identb": idenbf,
            "maskd": maskd,
        })
    return in_maps


def kernel(h, position_ids, wq, wk, wv, wo):
    global LAST_RESULTS
    from concourse.bass_utils import run_bass_kernel_spmd

    if "nc" not in _CACHE:
        _CACHE["nc"] = _build_program()
    nc = _CACHE["nc"]

    in_maps = _pack_inputs(
        np.asarray(h, dtype=np.float32),
        np.asarray(position_ids),
        np.asarray(wq, dtype=np.float32),
        np.asarray(wk, dtype=np.float32),
        np.asarray(wv, dtype=np.float32),
        np.asarray(wo, dtype=np.float32),
    )

    trace = bool(int(os.environ.get("KERNEL_TRACE", "0")))
    res = run_bass_kernel_spmd(
        nc, in_maps, core_ids=list(range(NCORES)), trace=trace
    )
    LAST_RESULTS = res

    acc = np.zeros((S, HID), dtype=np.float32)
    for r in res.results:
        acc += r["out"].astype(np.float32)
    return acc


# revision 25
# speedup vs baseline: 1.0216x; 1.0033x over previous
"""Mixtral GQA attention block (B=1, S=2048, HID=4096, NH=32, NKV=8, HD=128),
8-way tensor-parallel over heads on trn2: each core owns 4 query heads + 1 KV
head (one GQA group), computes its partial output projection, host sums the
8 partials.

Device layout notes:
  - All matmul operands are staged transposed (contraction dim on partitions).
    Host pre-packs every tensor partition-major so DMAs are identity copies.
  - The Q/K/V and output projections run as fp8e4 DoubleRow matmuls with a
    3-term hi/lo error compensation: x@w ~= xh@wh + xh@wl + xl@wh. Pieces are
    packed [128, ktile, piece, N] with lhsT pieces (hi, lo) and rhs pieces
    (lo, hi); per ktile pair one DR instr covers both hi@hi products and one
    DR instr per ktile covers both cross products -> 0.75x the f16 PE cost
    at ~1e-3 accuracy. Tensors are pre-scaled into fp8's normal range
    (h x32, weights x1024, attn x16) and the product descale is folded into
    the PSUM eviction ops.
  - Scores are computed transposed (S^T[k,q]) in f16 so the exp'd tiles
    directly serve as lhsT for the P@V matmul; softmax denominator comes from
    an appended ones-column on V; no max-subtraction. Causal masking of the
    diagonal-band blocks is a -60000 bias added INTO the scores PSUM group
    via an identity-lhsT matmul, so exp needs no follow-up mask multiply.
  - RoPE is applied in the transposed orientation; the half-swap crosses
    partitions and runs as two partition-offset SBUF->SBUF DMAs.
  - The attention drain loop is Activation-bound (exp), so independent PE
    work is software-pipelined into it through a pending-work queue:
    chunk 0 streams the q1..q3 projections through attention(0); chunks
    1..3 stream the previous chunk's output projection. Chunk 0's k/v/q0
    projections are 3-way interleaved per ktile pair so the PE tracks the
    cold-start h DMA stream; DMAs are ordered by first use.
"""

import math
import os
import sys

import numpy as np

sys.path.insert(0, "/opt/trn_rl_repo")

import concourse.bass as bass
import concourse.tile as tile
from concourse import bacc
from concourse import mybir

S = 2048
HID = 4096
NH, NKV, HD = 32, 8, 128
NCORES = 8
QH = NH // NCORES      # 4 query heads per core
ND = HID // 128        # 32 contraction chunks
NI = S // 512          # 4 q-chunks of 512
NJ = S // 128          # 16 k-tiles of 128
SCALE = 1.0 / math.sqrt(HD)

F16 = mybir.dt.float16
BF16 = mybir.dt.bfloat16
F32 = mybir.dt.float32
F8 = mybir.dt.float8e4
DR = mybir.MatmulPerfMode.DoubleRow

SH = 32.0        # fp8 pre-scale on h
SW = 1024.0      # fp8 pre-scale on wq/wk/wv/wo
SA = 16.0        # fp8 pre-scale on attention output
DESCALE_QKV = float(1.0 / (SH * SW))
DESCALE_O = float(1.0 / (SA * SW))
MASK_NEG = -60000.0

_CACHE = {}
LAST_RESULTS = None


def _dr_ops(w_ap, h_ap, nd):
    """(lhsT, rhs) DR operand pairs for the 3-term hi/lo product over `nd`
    ktiles, in ktile-streaming order. w pieces are (hi, lo), h (lo, hi)."""
    ops = []
    for a in range(0, nd, 2):
        ops.append((w_ap(a, 0, False), h_ap(a, 1, False)))      # hi@hi pair
        ops.append((w_ap(a, None, True), h_ap(a, None, True)))  # cross a
        ops.append((w_ap(a + 1, None, True), h_ap(a + 1, None, True)))
    return ops


def _build_program():
    nc = bacc.Bacc()

    h4 = nc.declare_dram_parameter("h4", [128, ND, 2, S], F8, isOutput=False)
    wq4 = nc.declare_dram_parameter("wq4", [128, QH, ND, 2, 128], F8,
                                    isOutput=False)
    wk4 = nc.declare_dram_parameter("wk4", [128, ND, 2, 128], F8, isOutput=False)
    wv4 = nc.declare_dram_parameter("wv4", [128, ND, 2, 128], F8, isOutput=False)
    wo4 = nc.declare_dram_parameter("wo4", [128, QH, 2, HID], F8, isOutput=False)
    cosd = nc.declare_dram_parameter("cosd", [128, S], F16, isOutput=False)
    identd = nc.declare_dram_parameter("identd", [128, 128], F16, isOutput=False)
    identb = nc.declare_dram_parameter("identb", [128, 128], BF16, isOutput=False)
    maskd = nc.declare_dram_parameter("maskd", [128, 128], F16, isOutput=False)
    sind = nc.declare_dram_parameter("sind", [128, S], F16, isOutput=False)
    out = nc.declare_dram_parameter("out", [S, HID], F16, isOutput=True)

    with tile.TileContext(nc) as tc:
        with (
            tc.tile_pool(name="consts", bufs=1) as consts,
            tc.tile_pool(name="hpool", bufs=10) as hpool,
            tc.tile_pool(name="ptpool", bufs=5) as ptpool,
            tc.tile_pool(name="rtmp", bufs=3) as rtmp,
            tc.tile_pool(name="small", bufs=8) as small,
            tc.tile_pool(name="orow", bufs=5) as orowp,
            tc.tile_pool(name="pproj", bufs=2, space="PSUM") as pproj,
            tc.tile_pool(name="pwork", bufs=2, space="PSUM") as pwork,
            tc.tile_pool(name="popsum", bufs=4, space="PSUM") as popsum,
        ):
            wq_sb = consts.tile([128, QH, ND, 2, 128], F8)
            cos_sb = consts.tile([128, S], F16)
            sin_sb = consts.tile([128, S], F16)
            wk_sb = consts.tile([128, ND, 2, 128], F8)
            wv_sb = consts.tile([128, ND, 2, 128], F8)
            wo_sb = consts.tile([128, QH, 2, HID], F8)
            ident16 = consts.tile([128, 128], F16)
            identbf = consts.tile([128, 128], BF16)
            maskb = consts.tile([128, 128], F16)

            hts0 = []
            for dq in range(8):
                t_h = hpool.tile([128, 4, 2, 512], F8, tag="ht",
                                 name=f"ht_0_{dq}")
                hts0.append(t_h)

            def ht_dma(tiles, I, dq):
                nc.sync.dma_start(
                    out=tiles[dq],
                    in_=h4[:, dq * 4:(dq + 1) * 4, :, I * 512:(I + 1) * 512],
                )

            def wq_dma(t, half):
                dsl = slice(half * 16, (half + 1) * 16)
                nc.sync.dma_start(out=wq_sb[:, t, dsl, :, :],
                                  in_=wq4[:, t, dsl, :, :])

            # Startup DMA order = first-use order: k/v/q0 weights lead the
            # cold h stream; tables next; q1..q3 heads land during
            # attention(0); wo column blocks land before attention(1).
            nc.sync.dma_start(out=wk_sb[:, 0:8, :, :], in_=wk4[:, 0:8, :, :])
            nc.sync.dma_start(out=hts0[0][:, 0:2, :, :],
                              in_=h4[:, 0:2, :, 0:512])
            nc.sync.dma_start(out=wv_sb[:, 0:8, :, :], in_=wv4[:, 0:8, :, :])
            nc.sync.dma_start(out=hts0[0][:, 2:4, :, :],
                              in_=h4[:, 2:4, :, 0:512])
            nc.sync.dma_start(out=wq_sb[:, 0, 0:8, :, :],
                              in_=wq4[:, 0, 0:8, :, :])
            nc.sync.dma_start(out=wk_sb[:, 8:16, :, :], in_=wk4[:, 8:16, :, :])
            nc.sync.dma_start(out=wv_sb[:, 8:16, :, :], in_=wv4[:, 8:16, :, :])
            nc.sync.dma_start(out=wq_sb[:, 0, 8:16, :, :],
                              in_=wq4[:, 0, 8:16, :, :])
            nc.sync.dma_start(out=wk_sb[:, 16:32, :, :], in_=wk4[:, 16:32, :, :])
            ht_dma(hts0, 0, 1)
            nc.sync.dma_start(out=wv_sb[:, 16:32, :, :], in_=wv4[:, 16:32, :, :])
            ht_dma(hts0, 0, 2)
            wq_dma(0, 1)
            ht_dma(hts0, 0, 3)
            for dq in range(4, 8):
                ht_dma(hts0, 0, dq)
            nc.sync.dma_start(out=cos_sb, in_=cosd[:, :])
            nc.sync.dma_start(out=sin_sb, in_=sind[:, :])
            nc.sync.dma_start(out=identbf, in_=identb[:, :])
            nc.sync.dma_start(out=maskb, in_=maskd[:, :])
            nc.sync.dma_start(out=ident16, in_=identd[:, :])
            for t in range(1, QH):
                wq_dma(t, 0)
                wq_dma(t, 1)

            qT = consts.tile([128, QH, S], F16)    # roped q, transposed
            kT = consts.tile([128, S], F16)        # roped k, transposed
            # V' tiles: per k-tile j, [128 tokens, 128 ch + ones column]
            vA = consts.tile([128, NJ, 132], BF16)
            for j in range(NJ):
                nc.vector.memset(vA[:, j, 128:129], 1.0)
            # attn output pieces for the fp8 O-projection: [hd, head, piece, S]
            attn4 = consts.tile([128, QH, 2, S], F8)

            def w_ap_of(w_tile, t=None):
                if t is None:
                    def w_ap(a, piece, single):
                        if single:
                            return w_tile[:, a, 0:2, :]
                        return w_tile[:, a:a + 2, piece, :]
                else:
                    def w_ap(a, piece, single):
                        if single:
                            return w_tile[:, t, a, 0:2, :]
                        return w_tile[:, t, a:a + 2, piece, :]
                return w_ap

            def h_ap_of(hts):
                def h_ap(a, piece, single):
                    if single:
                        return hts[a // 4][:, a % 4, 0:2, :]
                    return hts[a // 4][:, a % 4:a % 4 + 2, piece, :]
                return h_ap

            def emit_chain(ps, ops, lo, hi_i):
                for i in range(lo, hi_i):
                    nc.tensor.matmul(
                        ps, ops[i][0], ops[i][1],
                        start=(i == 0), stop=(i == len(ops) - 1),
                        perf_mode=DR,
                    )

            def rope_into(ps, dst, nsl):
                """ps: PSUM [128, 512] f32 pre-rope (transposed layout,
                scaled by SH*SW). dst: f16 SBUF slice. The half-swap
                crosses partitions -> two partition-offset DMAs."""
                cpy = rtmp.tile([128, 512], F16, tag="ropecpy")
                nc.scalar.mul(cpy, ps, DESCALE_QKV)
                sw = rtmp.tile([128, 512], F16, tag="ropesw")
                nc.sync.dma_start(out=sw[0:64, :], in_=cpy[64:128, :])
                nc.sync.dma_start(out=sw[64:128, :], in_=cpy[0:64, :])
                tmp2 = rtmp.tile([128, 512], F16, tag="ropecos")
                nc.gpsimd.tensor_mul(tmp2, cpy, cos_sb[:, nsl])
                nc.gpsimd.tensor_mul(sw, sw, sin_sb[:, nsl])
                nc.vector.tensor_add(dst, tmp2, sw)

            def v_stage(v_ps, I):
                vt_sb = small.tile([128, 512], BF16, tag="vt")
                nc.vector.tensor_scalar_mul(vt_sb, v_ps, DESCALE_QKV)
                for jj in range(4):
                    tps = pwork.tile([128, 128], BF16, tag="work")
                    nc.tensor.transpose(
                        tps, vt_sb[:, jj * 128:(jj + 1) * 128], identbf
                    )
                    nc.vector.tensor_copy(vA[:, 4 * I + jj, 0:128], tps)

            # ---- O-projection emission units ------------------------------
            def make_oproj_unit(I, il, qtr, split_dma=False, act_evict=False):
                i_abs = 4 * I + il
                tok = slice(i_abs * 128, (i_abs + 1) * 128)

                def a_ap(a, piece, single):
                    if single:
                        return attn4[:, a, 0:2, tok]
                    return attn4[:, a:a + 2, piece, tok]

                def emit():
                    orow = orowp.tile([128, 1024], F16, tag="orow")
                    for mc in range(2):
                        mq = qtr * 1024 + mc * 512

                        def w_ap(a, piece, single, mq=mq):
                            if single:
                                return wo_sb[:, a, 0:2, mq:mq + 512]
                            return wo_sb[:, a:a + 2, piece, mq:mq + 512]

                        op_ps = pproj.tile([128, 512], F32, tag="proj")
                        ops = _dr_ops(a_ap, w_ap, QH)
                        emit_chain(op_ps, ops, 0, len(ops))
                        osl = orow[:, mc * 512:(mc + 1) * 512]
                        if act_evict and mc == 1:
                            nc.scalar.mul(osl, op_ps, DESCALE_O)
                        else:
                            nc.vector.tensor_scalar_mul(osl, op_ps, DESCALE_O)
                        if split_dma:
                            nc.sync.dma_start(
                                out=out[i_abs * 128:(i_abs + 1) * 128,
                                        mq:mq + 512],
                                in_=osl,
                            )
                    if not split_dma:
                        nc.sync.dma_start(
                            out=out[i_abs * 128:(i_abs + 1) * 128,
                                    qtr * 1024:(qtr + 1) * 1024],
                            in_=orow,
                        )
                return emit

            # ---- chunk 0 cold start: 3-way interleaved k/v/q0 -------------
            nsl0 = slice(0, 512)
            hap0 = h_ap_of(hts0)
            k_ps = pproj.tile([128, 512], F32, tag="proj")
            v_ps = pproj.tile([128, 512], F32, tag="proj")
            q0_ps = popsum.tile([128, 512], F32, tag="opsum", name="q_ps_0_0")
            k_ops = _dr_ops(w_ap_of(wk_sb), hap0, ND)
            v_ops = _dr_ops(w_ap_of(wv_sb), hap0, ND)
            q0_ops = _dr_ops(w_ap_of(wq_sb, 0), hap0, ND)
            for pr in range(ND // 2):
                emit_chain(k_ps, k_ops, 3 * pr, 3 * pr + 3)
                emit_chain(v_ps, v_ops, 3 * pr, 3 * pr + 3)
                emit_chain(q0_ps, q0_ops, 3 * pr, 3 * pr + 3)
            rope_into(k_ps, kT[:, nsl0], nsl0)
            v_stage(v_ps, 0)
            rope_into(q0_ps, qT[:, 0, nsl0], nsl0)

            # chunk-0 pending work: q1..q3 projections streamed through
            # attention(0)'s drains. The chunk-1 h prefetch is emitted right
            # after the LAST q3 group reading each chunk-0 sub-tile (the
            # hpool buffer reuse must come after every reader is emitted),
            # and the wo column blocks trail at the end.
            pending = []
            q_state = {}
            hts_next = []

            def make_q_group(t, pr, hts, nsl):
                def emit():
                    if t not in q_state:
                        q_state[t] = {
                            "ps": pproj.tile([128, 512], F32, tag="proj",
                                             name=f"q_ps_0_{t}"),
                            "ops": _dr_ops(w_ap_of(wq_sb, t), h_ap_of(hts),
                                           ND),
                        }
                    st = q_state[t]
                    emit_chain(st["ps"], st["ops"], 3 * pr, 3 * pr + 3)
                    if pr == ND // 2 - 1:
                        rope_into(st["ps"], qT[:, t, nsl], nsl)
                        del q_state[t]
                return emit

            def make_h1_prefetch(dq):
                def emit():
                    t_h = hpool.tile([128, 4, 2, 512], F8, tag="ht",
                                     name=f"ht_1_{dq}")
                    assert len(hts_next) == dq
                    hts_next.append(t_h)
                    ht_dma(hts_next, 1, dq)
                return emit

            def make_wo_dma(cb):
                def emit():
                    csl = slice(cb * 1024, (cb + 1) * 1024)
                    nc.sync.dma_start(out=wo_sb[:, :, :, csl],
                                      in_=wo4[:, :, :, csl])
                return emit

            # per-head buckets: bucket t is fully emitted during head t's
            # drains, so head t+1's scores (which need qT[t+1]) are never
            # enqueued ahead of the projection work they depend on.
            pending = [[], [], [], []]
            for t in range(1, QH):
                for pr in range(ND // 2):
                    pending[t - 1].append(make_q_group(t, pr, hts0, nsl0))
                    if t == QH - 1 and pr % 2 == 1:
                        pending[t - 1].append(make_h1_prefetch(pr // 2))
            for cb in range(4):
                pending[3].append(make_wo_dma(cb))

            hts_cur = hts_next
            for I in range(NI):
                nsl = slice(I * 512, (I + 1) * 512)
                hts = hts_cur

                if I > 0:
                    # ---- projections (h prefetched; PE-bound) ------------
                    k_ps = pproj.tile([128, 512], F32, tag="proj")
                    emit_chain(k_ps, _dr_ops(w_ap_of(wk_sb), h_ap_of(hts),
                                             ND), 0, 3 * (ND // 2))
                    rope_into(k_ps, kT[:, nsl], nsl)

                    v_ps = pproj.tile([128, 512], F32, tag="proj")
                    emit_chain(v_ps, _dr_ops(w_ap_of(wv_sb), h_ap_of(hts),
                                             ND), 0, 3 * (ND // 2))
                    v_stage(v_ps, I)

                    for t in range(QH):
                        q_ps = pproj.tile([128, 512], F32, tag="proj")
                        emit_chain(q_ps, _dr_ops(w_ap_of(wq_sb, t),
                                                 h_ap_of(hts), ND),
                                   0, 3 * (ND // 2))
                        rope_into(q_ps, qT[:, t, nsl], nsl)

                    # prefetch next pass's h slice
                    if I + 1 < NI:
                        hts_cur = []
                        for dq in range(8):
                            t_h = hpool.tile([128, 4, 2, 512], F8, tag="ht",
                                             name=f"ht_{I + 1}_{dq}")
                            hts_cur.append(t_h)
                            ht_dma(hts_cur, I + 1, dq)

                # ---- attention for q-chunk I, with pending PE work
                # interleaved into the drain loop ----------------------
                njt = 4 * I + 4   # k-tiles in causal range of this chunk
                state = {}

                def start_head(t, I=I):
                    o_ps = [
                        popsum.tile([128, 132], F32, tag="opsum",
                                    name=f"o_ps_{I}_{t}_{il}")
                        for il in range(4)
                    ]
                    return {"o_ps": o_ps}

                def finalize_il(t, il, I=I):
                    o_ps = state[t]["o_ps"]
                    i_abs = 4 * I + il
                    recip = small.tile([128, 1], F32, tag="recip")
                    nc.vector.reciprocal(recip, o_ps[il][:, 128:129])
                    # osc carries the fp8 pre-scale SA so the casts are
                    # plain copy/sub ops that can run on the Pool engine
                    osc = small.tile([128, 128], F16, tag="osc")
                    nc.vector.tensor_scalar(
                        osc, o_ps[il][:, 0:128], recip, SA,
                        mybir.AluOpType.mult, mybir.AluOpType.mult,
                    )
                    tps = popsum.tile([128, 132], F16, tag="opsum",
                                      name=f"tps_{I}_{t}_{il}")
                    nc.tensor.transpose(tps[:, 0:128], osc, ident16)
                    att16 = small.tile([128, 128], F16, tag="att16")
                    nc.vector.tensor_copy(att16, tps[:, 0:128])
                    tok = slice(i_abs * 128, (i_abs + 1) * 128)
                    nc.gpsimd.tensor_copy(attn4[:, t, 0, tok], att16)
                    nc.gpsimd.tensor_sub(
                        attn4[:, t, 1, tok], att16, attn4[:, t, 0, tok]
                    )

                def drain(t, jprev, s_ps, I=I):
                    m = jprev - 4 * I
                    q_off = 128 * m if m > 0 else 0
                    pt = ptpool.tile([128, 512], BF16, tag="pt")
                    nc.scalar.activation(
                        pt[:, q_off:512], s_ps[:, q_off:512],
                        mybir.ActivationFunctionType.Exp,
                        scale=SCALE,
                    )
                    o_ps = state[t]["o_ps"]
                    for il in range(4):
                        i_abs = 4 * I + il
                        if jprev <= i_abs:
                            nc.tensor.matmul(
                                o_ps[il][:, 0:129],
                                pt[:, il * 128:(il + 1) * 128],
                                vA[:, jprev, 0:129],
                                start=(jprev == 0),
                                stop=(jprev == i_abs),
                            )
                            if jprev == i_abs:
                                finalize_il(t, il)

                prev = None
                for t in range(QH):
                    state[t] = start_head(t)
                    bucket = pending[t]
                    bucket_total = len(bucket)
                    bucket_done = 0
                    for j in range(njt):
                        mj = j - 4 * I
                        q_off = 128 * mj if mj > 0 else 0
                        s_ps = pwork.tile([128, 512], F32, tag="work")
                        diag = mj >= 0
                        nc.tensor.matmul(
                            s_ps[:, q_off:512],
                            kT[:, j * 128:(j + 1) * 128],
                            qT[:, t, I * 512 + q_off:(I + 1) * 512],
                            start=True, stop=not diag,
                        )
                        if diag:
                            # -60000 causal bias onto the diagonal block
                            nc.tensor.matmul(
                                s_ps[:, q_off:q_off + 128],
                                ident16, maskb,
                                start=False, stop=True,
                                skip_group_check=True,
                            )
                        if prev is not None:
                            tp_, jp_, sp_ = prev
                            drain(tp_, jp_, sp_)
                            target = (j + 1) * bucket_total // njt
                            while bucket and bucket_done < target:
                                bucket.pop(0)()
                                bucket_done += 1
                        elif bucket:
                            # prime the PE while the first exp runs
                            bucket.pop(0)()
                            bucket_done += 1
                        prev = (t, j, s_ps)
                    for fn in bucket:
                        fn()
                tp_, jp_, sp_ = prev
                drain(tp_, jp_, sp_)
                last = I == NI - 1
                units = [make_oproj_unit(I, il, qtr, split_dma=last,
                                          act_evict=last)
                         for il in range(4) for qtr in range(4)]
                pending = [units[0:4], units[4:8], units[8:12], units[12:16]]

            for bucket in pending:
                for fn in bucket:
                    fn()
    nc.finalize()
    return nc


def _hilo(x, scale):
    """Split x*scale into fp8e4 (hi, lo) pieces."""
    import ml_dtypes

    xs = (x * scale).astype(np.float32)
    hi = xs.astype(ml_dtypes.float8_e4m3)
    lo = (xs - hi.astype(np.float32)).astype(ml_dtypes.float8_e4m3)
    return hi, lo


def _pack4(xT, scale, nd, ncols, lhs):
    """Pack xT [nd*128, ncols] into [128, nd, 2, ncols] fp8 pieces.
    lhs=True -> pieces (hi, lo); False -> (lo, hi)."""
    import ml_dtypes

    hi, lo = _hilo(xT, scale)
    outp = np.empty((128, nd, 2, ncols), dtype=ml_dtypes.float8_e4m3)
    h3 = hi.reshape(nd, 128, ncols)
    l3 = lo.reshape(nd, 128, ncols)
    if lhs:
        outp[:, :, 0, :] = h3.transpose(1, 0, 2)
        outp[:, :, 1, :] = l3.transpose(1, 0, 2)
    else:
        outp[:, :, 0, :] = l3.transpose(1, 0, 2)
        outp[:, :, 1, :] = h3.transpose(1, 0, 2)
    return outp


def _pack_inputs(h, position_ids, wq, wk, wv, wo):
    """Host-side shard + transpose + fp8 hi/lo split. Per-core input maps."""
    import ml_dtypes

    h4 = _pack4(np.ascontiguousarray(h.T), SH, ND, S, lhs=False)

    # RoPE tables in transposed orientation, halves duplicated / sign-folded.
    inv = 1.0 / (1e6 ** (np.arange(0, HD, 2, dtype=np.float64) / HD))
    fr = position_ids.astype(np.float64)[None, :] * inv[:, None]   # [64, S]
    cosT = np.cos(fr).astype(np.float16)
    sinT = np.sin(fr).astype(np.float16)
    cosd = np.concatenate([cosT, cosT], axis=0)                    # [128, S]
    sind = np.concatenate([-sinT, sinT], axis=0)
    iden16 = np.eye(128, dtype=np.float16)
    idenbf = np.eye(128).astype(ml_dtypes.bfloat16)
    p_i = np.arange(128)[:, None]
    f_i = np.arange(128)[None, :]
    maskd = np.where(f_i - p_i >= 0, 0.0, MASK_NEG).astype(np.float16)

    in_maps = []
    for c in range(NCORES):
        wq_c = wq[c * 512:(c + 1) * 512, :]          # [512, HID]
        wk_c = wk[c * 128:(c + 1) * 128, :]
        wv_c = wv[c * 128:(c + 1) * 128, :]
        wo_c = wo[:, c * 512:(c + 1) * 512]          # [HID, 512]
        woT = np.ascontiguousarray(wo_c.T)           # [512, HID]
        wqT = np.ascontiguousarray(wq_c.T)           # [HID, 512]
        wq4 = np.stack(
            [_pack4(np.ascontiguousarray(wqT[:, t * 128:(t + 1) * 128]),
                    SW, ND, 128, lhs=True) for t in range(QH)],
            axis=1,
        )                                            # [128, QH, ND, 2, 128]
        in_maps.append({
            "h4": h4,
            "wq4": wq4,
            "wk4": _pack4(np.ascontiguousarray(wk_c.T), SW, ND, 128, lhs=True),
            "wv4": _pack4(np.ascontiguousarray(wv_c.T), SW, ND, 128, lhs=True),
            "wo4": _pack4(woT, SW, QH, HID, lhs=False),
            "cosd": cosd,
            "sind": sind,
            "identd": iden16,
            "identb": idenbf,
            "maskd": maskd,
        })
    return in_maps


def kernel(h, position_ids, wq, wk, wv, wo):
    global LAST_RESULTS
    from concourse.bass_utils import run_bass_kernel_spmd

    if "nc" not in _CACHE:
        _CACHE["nc"] = _build_program()
    nc = _CACHE["nc"]

    in_maps = _pack_inputs(
        np.asarray(h, dtype=np.float32),
        np.asarray(position_ids),
        np.asarray(wq, dtype=np.float32),
        np.asarray(wk, dtype=np.float32),
        np.asarray(wv, dtype=np.float32),
        np.asarray(wo, dtype=np.float32),
    )

    trace = bool(int(os.environ.get("KERNEL_TRACE", "0")))
    res = run_bass_kernel_spmd(
        nc, in_maps, core_ids=list(range(NCORES)), trace=trace
    )
    LAST_RESULTS = res

    acc = np.zeros((S, HID), dtype=np.float32)
    for r in res.results:
        acc += r["out"].astype(np.float32)
    return acc


# revision 26
# speedup vs baseline: 1.0217x; 1.0001x over previous
"""Mixtral GQA attention block (B=1, S=2048, HID=4096, NH=32, NKV=8, HD=128),
8-way tensor-parallel over heads on trn2: each core owns 4 query heads + 1 KV
head (one GQA group), computes its partial output projection, host sums the
8 partials.

Device layout notes:
  - All matmul operands are staged transposed (contraction dim on partitions).
    Host pre-packs every tensor partition-major so DMAs are identity copies.
  - The Q/K/V and output projections run as fp8e4 DoubleRow matmuls with a
    3-term hi/lo error compensation: x@w ~= xh@wh + xh@wl + xl@wh. Pieces are
    packed [128, ktile, piece, N] with lhsT pieces (hi, lo) and rhs pieces
    (lo, hi); per ktile pair one DR instr covers both hi@hi products and one
    DR instr per ktile covers both cross products -> 0.75x the f16 PE cost
    at ~1e-3 accuracy. Tensors are pre-scaled into fp8's normal range
    (h x32, weights x1024, attn x16) and the product descale is folded into
    the PSUM eviction ops.
  - Scores are computed transposed (S^T[k,q]) in f16 so the exp'd tiles
    directly serve as lhsT for the P@V matmul; softmax denominator comes from
    an appended ones-column on V; no max-subtraction. Causal masking of the
    diagonal-band blocks is a -60000 bias added INTO the scores PSUM group
    via an identity-lhsT matmul, so exp needs no follow-up mask multiply.
  - RoPE is applied in the transposed orientation; the half-swap crosses
    partitions and runs as two partition-offset SBUF->SBUF DMAs.
  - The attention drain loop is Activation-bound (exp), so independent PE
    work is software-pipelined into it through a pending-work queue:
    chunk 0 streams the q1..q3 projections through attention(0); chunks
    1..3 stream the previous chunk's output projection. Chunk 0's k/v/q0
    projections are 3-way interleaved per ktile pair so the PE tracks the
    cold-start h DMA stream; DMAs are ordered by first use.
"""

import math
import os
import sys

import numpy as np

sys.path.insert(0, "/opt/trn_rl_repo")

import concourse.bass as bass
import concourse.tile as tile
from concourse import bacc
from concourse import mybir

S = 2048
HID = 4096
NH, NKV, HD = 32, 8, 128
NCORES = 8
QH = NH // NCORES      # 4 query heads per core
ND = HID // 128        # 32 contraction chunks
NI = S // 512          # 4 q-chunks of 512
NJ = S // 128          # 16 k-tiles of 128
SCALE = 1.0 / math.sqrt(HD)

F16 = mybir.dt.float16
BF16 = mybir.dt.bfloat16
F32 = mybir.dt.float32
F8 = mybir.dt.float8e4
DR = mybir.MatmulPerfMode.DoubleRow

SH = 32.0        # fp8 pre-scale on h
SW = 1024.0      # fp8 pre-scale on wq/wk/wv/wo
SA = 16.0        # fp8 pre-scale on attention output
DESCALE_QKV = float(1.0 / (SH * SW))
DESCALE_O = float(1.0 / (SA * SW))
MASK_NEG = -60000.0

_CACHE = {}
LAST_RESULTS = None


def _dr_ops(w_ap, h_ap, nd):
    """(lhsT, rhs) DR operand pairs for the 3-term hi/lo product over `nd`
    ktiles, in ktile-streaming order. w pieces are (hi, lo), h (lo, hi)."""
    ops = []
    for a in range(0, nd, 2):
        ops.append((w_ap(a, 0, False), h_ap(a, 1, False)))      # hi@hi pair
        ops.append((w_ap(a, None, True), h_ap(a, None, True)))  # cross a
        ops.append((w_ap(a + 1, None, True), h_ap(a + 1, None, True)))
    return ops


def _build_program():
    nc = bacc.Bacc()

    h4 = nc.declare_dram_parameter("h4", [128, ND, 2, S], F8, isOutput=False)
    wq4 = nc.declare_dram_parameter("wq4", [128, QH, ND, 2, 128], F8,
                                    isOutput=False)
    wk4 = nc.declare_dram_parameter("wk4", [128, ND, 2, 128], F8, isOutput=False)
    wv4 = nc.declare_dram_parameter("wv4", [128, ND, 2, 128], F8, isOutput=False)
    wo4 = nc.declare_dram_parameter("wo4", [128, QH, 2, HID], F8, isOutput=False)
    cosd = nc.declare_dram_parameter("cosd", [128, S], F16, isOutput=False)
    identd = nc.declare_dram_parameter("identd", [128, 128], F16, isOutput=False)
    identb = nc.declare_dram_parameter("identb", [128, 128], BF16, isOutput=False)
    maskd = nc.declare_dram_parameter("maskd", [128, 128], F16, isOutput=False)
    sind = nc.declare_dram_parameter("sind", [128, S], F16, isOutput=False)
    out = nc.declare_dram_parameter("out", [S, HID], F16, isOutput=True)

    with tile.TileContext(nc) as tc:
        with (
            tc.tile_pool(name="consts", bufs=1) as consts,
            tc.tile_pool(name="hpool", bufs=10) as hpool,
            tc.tile_pool(name="ptpool", bufs=5) as ptpool,
            tc.tile_pool(name="rtmp", bufs=3) as rtmp,
            tc.tile_pool(name="small", bufs=8) as small,
            tc.tile_pool(name="orow", bufs=5) as orowp,
            tc.tile_pool(name="pproj", bufs=2, space="PSUM") as pproj,
            tc.tile_pool(name="pwork", bufs=2, space="PSUM") as pwork,
            tc.tile_pool(name="popsum", bufs=4, space="PSUM") as popsum,
        ):
            wq_sb = consts.tile([128, QH, ND, 2, 128], F8)
            cos_sb = consts.tile([128, S], F16)
            sin_sb = consts.tile([128, S], F16)
            wk_sb = consts.tile([128, ND, 2, 128], F8)
            wv_sb = consts.tile([128, ND, 2, 128], F8)
            wo_sb = consts.tile([128, QH, 2, HID], F8)
            ident16 = consts.tile([128, 128], F16)
            identbf = consts.tile([128, 128], BF16)
            maskb = consts.tile([128, 128], F16)

            hts0 = []
            for dq in range(8):
                t_h = hpool.tile([128, 4, 2, 512], F8, tag="ht",
                                 name=f"ht_0_{dq}")
                hts0.append(t_h)

            def ht_dma(tiles, I, dq):
                nc.sync.dma_start(
                    out=tiles[dq],
                    in_=h4[:, dq * 4:(dq + 1) * 4, :, I * 512:(I + 1) * 512],
                )

            def wq_dma(t, half):
                dsl = slice(half * 16, (half + 1) * 16)
                nc.sync.dma_start(out=wq_sb[:, t, dsl, :, :],
                                  in_=wq4[:, t, dsl, :, :])

            # Startup DMA order = first-use order: k/v/q0 weights lead the
            # cold h stream; tables next; q1..q3 heads land during
            # attention(0); wo column blocks land before attention(1).
            nc.sync.dma_start(out=wk_sb[:, 0:8, :, :], in_=wk4[:, 0:8, :, :])
            nc.sync.dma_start(out=hts0[0][:, 0:2, :, :],
                              in_=h4[:, 0:2, :, 0:512])
            nc.sync.dma_start(out=wv_sb[:, 0:8, :, :], in_=wv4[:, 0:8, :, :])
            nc.sync.dma_start(out=hts0[0][:, 2:4, :, :],
                              in_=h4[:, 2:4, :, 0:512])
            nc.sync.dma_start(out=wq_sb[:, 0, 0:8, :, :],
                              in_=wq4[:, 0, 0:8, :, :])
            nc.sync.dma_start(out=wk_sb[:, 8:16, :, :], in_=wk4[:, 8:16, :, :])
            nc.sync.dma_start(out=wv_sb[:, 8:16, :, :], in_=wv4[:, 8:16, :, :])
            nc.sync.dma_start(out=wq_sb[:, 0, 8:16, :, :],
                              in_=wq4[:, 0, 8:16, :, :])
            nc.sync.dma_start(out=wk_sb[:, 16:32, :, :], in_=wk4[:, 16:32, :, :])
            ht_dma(hts0, 0, 1)
            nc.sync.dma_start(out=wv_sb[:, 16:32, :, :], in_=wv4[:, 16:32, :, :])
            ht_dma(hts0, 0, 2)
            wq_dma(0, 1)
            ht_dma(hts0, 0, 3)
            for dq in range(4, 8):
                ht_dma(hts0, 0, dq)
            nc.sync.dma_start(out=cos_sb, in_=cosd[:, :])
            nc.sync.dma_start(out=sin_sb, in_=sind[:, :])
            nc.sync.dma_start(out=identbf, in_=identb[:, :])
            nc.sync.dma_start(out=maskb, in_=maskd[:, :])
            nc.sync.dma_start(out=ident16, in_=identd[:, :])
            for t in range(1, QH):
                wq_dma(t, 0)
                wq_dma(t, 1)

            qT = consts.tile([128, QH, S], F16)    # roped q, transposed
            kT = consts.tile([128, S], F16)        # roped k, transposed
            # V' tiles: per k-tile j, [128 tokens, 128 ch + ones column]
            vA = consts.tile([128, NJ, 132], BF16)
            for j in range(NJ):
                nc.vector.memset(vA[:, j, 128:129], 1.0)
            # attn output pieces for the fp8 O-projection: [hd, head, piece, S]
            attn4 = consts.tile([128, QH, 2, S], F8)

            def w_ap_of(w_tile, t=None):
                if t is None:
                    def w_ap(a, piece, single):
                        if single:
                            return w_tile[:, a, 0:2, :]
                        return w_tile[:, a:a + 2, piece, :]
                else:
                    def w_ap(a, piece, single):
                        if single:
                            return w_tile[:, t, a, 0:2, :]
                        return w_tile[:, t, a:a + 2, piece, :]
                return w_ap

            def h_ap_of(hts):
                def h_ap(a, piece, single):
                    if single:
                        return hts[a // 4][:, a % 4, 0:2, :]
                    return hts[a // 4][:, a % 4:a % 4 + 2, piece, :]
                return h_ap

            def emit_chain(ps, ops, lo, hi_i):
                for i in range(lo, hi_i):
                    nc.tensor.matmul(
                        ps, ops[i][0], ops[i][1],
                        start=(i == 0), stop=(i == len(ops) - 1),
                        perf_mode=DR,
                    )

            def rope_into(ps, dst, nsl):
                """ps: PSUM [128, 512] f32 pre-rope (transposed layout,
                scaled by SH*SW). dst: f16 SBUF slice. The half-swap
                crosses partitions -> two partition-offset DMAs."""
                cpy = rtmp.tile([128, 512], F16, tag="ropecpy")
                nc.scalar.mul(cpy, ps, DESCALE_QKV)
                sw = rtmp.tile([128, 512], F16, tag="ropesw")
                nc.sync.dma_start(out=sw[0:64, :], in_=cpy[64:128, :])
                nc.sync.dma_start(out=sw[64:128, :], in_=cpy[0:64, :])
                tmp2 = rtmp.tile([128, 512], F16, tag="ropecos")
                nc.gpsimd.tensor_mul(tmp2, cpy, cos_sb[:, nsl])
                nc.gpsimd.tensor_mul(sw, sw, sin_sb[:, nsl])
                nc.vector.tensor_add(dst, tmp2, sw)

            def v_stage(v_ps, I):
                vt_sb = small.tile([128, 512], BF16, tag="vt")
                nc.vector.tensor_scalar_mul(vt_sb, v_ps, DESCALE_QKV)
                for jj in range(4):
                    tps = pwork.tile([128, 128], BF16, tag="work")
                    nc.tensor.transpose(
                        tps, vt_sb[:, jj * 128:(jj + 1) * 128], identbf
                    )
                    nc.vector.tensor_copy(vA[:, 4 * I + jj, 0:128], tps)

            # ---- O-projection emission units ------------------------------
            def make_oproj_unit(I, il, qtr, split_dma=False, act_evict=False):
                i_abs = 4 * I + il
                tok = slice(i_abs * 128, (i_abs + 1) * 128)

                def a_ap(a, piece, single):
                    if single:
                        return attn4[:, a, 0:2, tok]
                    return attn4[:, a:a + 2, piece, tok]

                def emit():
                    orow = orowp.tile([128, 1024], F16, tag="orow")
                    for mc in range(2):
                        mq = qtr * 1024 + mc * 512

                        def w_ap(a, piece, single, mq=mq):
                            if single:
                                return wo_sb[:, a, 0:2, mq:mq + 512]
                            return wo_sb[:, a:a + 2, piece, mq:mq + 512]

                        op_ps = pproj.tile([128, 512], F32, tag="proj")
                        ops = _dr_ops(a_ap, w_ap, QH)
                        emit_chain(op_ps, ops, 0, len(ops))
                        osl = orow[:, mc * 512:(mc + 1) * 512]
                        if act_evict and mc == 1:
                            nc.scalar.mul(osl, op_ps, DESCALE_O)
                        else:
                            nc.vector.tensor_scalar_mul(osl, op_ps, DESCALE_O)
                        if split_dma:
                            nc.sync.dma_start(
                                out=out[i_abs * 128:(i_abs + 1) * 128,
                                        mq:mq + 512],
                                in_=osl,
                            )
                    if not split_dma:
                        nc.sync.dma_start(
                            out=out[i_abs * 128:(i_abs + 1) * 128,
                                    qtr * 1024:(qtr + 1) * 1024],
                            in_=orow,
                        )
                return emit

            # ---- chunk 0 cold start: 3-way interleaved k/v/q0 -------------
            nsl0 = slice(0, 512)
            hap0 = h_ap_of(hts0)
            k_ps = pproj.tile([128, 512], F32, tag="proj")
            v_ps = pproj.tile([128, 512], F32, tag="proj")
            q0_ps = popsum.tile([128, 512], F32, tag="opsum", name="q_ps_0_0")
            k_ops = _dr_ops(w_ap_of(wk_sb), hap0, ND)
            v_ops = _dr_ops(w_ap_of(wv_sb), hap0, ND)
            q0_ops = _dr_ops(w_ap_of(wq_sb, 0), hap0, ND)
            for pr in range(ND // 2):
                emit_chain(k_ps, k_ops, 3 * pr, 3 * pr + 3)
                emit_chain(v_ps, v_ops, 3 * pr, 3 * pr + 3)
                emit_chain(q0_ps, q0_ops, 3 * pr, 3 * pr + 3)
            rope_into(k_ps, kT[:, nsl0], nsl0)
            v_stage(v_ps, 0)
            rope_into(q0_ps, qT[:, 0, nsl0], nsl0)

            # chunk-0 pending work: q1..q3 projections streamed through
            # attention(0)'s drains. The chunk-1 h prefetch is emitted right
            # after the LAST q3 group reading each chunk-0 sub-tile (the
            # hpool buffer reuse must come after every reader is emitted),
            # and the wo column blocks trail at the end.
            pending = []
            q_state = {}
            hts_next = []

            def make_q_group(t, pr, hts, nsl):
                def emit():
                    if t not in q_state:
                        q_state[t] = {
                            "ps": pproj.tile([128, 512], F32, tag="proj",
                                             name=f"q_ps_0_{t}"),
                            "ops": _dr_ops(w_ap_of(wq_sb, t), h_ap_of(hts),
                                           ND),
                        }
                    st = q_state[t]
                    emit_chain(st["ps"], st["ops"], 3 * pr, 3 * pr + 3)
                    if pr == ND // 2 - 1:
                        rope_into(st["ps"], qT[:, t, nsl], nsl)
                        del q_state[t]
                return emit

            def make_h1_prefetch(dq):
                def emit():
                    t_h = hpool.tile([128, 4, 2, 512], F8, tag="ht",
                                     name=f"ht_1_{dq}")
                    assert len(hts_next) == dq
                    hts_next.append(t_h)
                    ht_dma(hts_next, 1, dq)
                return emit

            def make_wo_dma(cb):
                def emit():
                    csl = slice(cb * 1024, (cb + 1) * 1024)
                    nc.sync.dma_start(out=wo_sb[:, :, :, csl],
                                      in_=wo4[:, :, :, csl])
                return emit

            # per-head buckets: bucket t is fully emitted during head t's
            # drains, so head t+1's scores (which need qT[t+1]) are never
            # enqueued ahead of the projection work they depend on.
            pending = [[], [], [], []]
            for t in range(1, QH):
                for pr in range(ND // 2):
                    pending[t - 1].append(make_q_group(t, pr, hts0, nsl0))
                    if t == QH - 1 and pr % 2 == 1:
                        pending[t - 1].append(make_h1_prefetch(pr // 2))
            for cb in range(4):
                pending[3].append(make_wo_dma(cb))

            hts_cur = hts_next
            for I in range(NI):
                nsl = slice(I * 512, (I + 1) * 512)
                hts = hts_cur

                if I > 0:
                    # ---- projections (h prefetched; PE-bound) ------------
                    k_ps = pproj.tile([128, 512], F32, tag="proj")
                    emit_chain(k_ps, _dr_ops(w_ap_of(wk_sb), h_ap_of(hts),
                                             ND), 0, 3 * (ND // 2))
                    rope_into(k_ps, kT[:, nsl], nsl)

                    v_ps = pproj.tile([128, 512], F32, tag="proj")
                    emit_chain(v_ps, _dr_ops(w_ap_of(wv_sb), h_ap_of(hts),
                                             ND), 0, 3 * (ND // 2))
                    v_stage(v_ps, I)

                    for t in range(QH):
                        q_ps = pproj.tile([128, 512], F32, tag="proj")
                        emit_chain(q_ps, _dr_ops(w_ap_of(wq_sb, t),
                                                 h_ap_of(hts), ND),
                                   0, 3 * (ND // 2))
                        rope_into(q_ps, qT[:, t, nsl], nsl)

                    # prefetch next pass's h slice
                    if I + 1 < NI:
                        hts_cur = []
                        for dq in range(8):
                            t_h = hpool.tile([128, 4, 2, 512], F8, tag="ht",
                                             name=f"ht_{I + 1}_{dq}")
                            hts_cur.append(t_h)
                            ht_dma(hts_cur, I + 1, dq)

                # ---- attention for q-chunk I, with pending PE work
                # interleaved into the drain loop ----------------------
                njt = 4 * I + 4   # k-tiles in causal range of this chunk
                state = {}

                def start_head(t, I=I):
                    o_ps = [
                        popsum.tile([128, 132], F32, tag="opsum",
                                    name=f"o_ps_{I}_{t}_{il}")
                        for il in range(4)
                    ]
                    return {"o_ps": o_ps}

                def finalize_il(t, il, I=I):
                    o_ps = state[t]["o_ps"]
                    i_abs = 4 * I + il
                    recip = small.tile([128, 1], F32, tag="recip")
                    nc.vector.reciprocal(recip, o_ps[il][:, 128:129])
                    # osc carries the fp8 pre-scale SA so the casts are
                    # plain copy/sub ops that can run on the Pool engine
                    osc = small.tile([128, 128], F16, tag="osc")
                    nc.vector.tensor_scalar(
                        osc, o_ps[il][:, 0:128], recip, SA,
                        mybir.AluOpType.mult, mybir.AluOpType.mult,
                    )
                    tps = popsum.tile([128, 132], F16, tag="opsum",
                                      name=f"tps_{I}_{t}_{il}")
                    nc.tensor.transpose(tps[:, 0:128], osc, ident16)
                    att16 = small.tile([128, 128], F16, tag="att16")
                    nc.vector.tensor_copy(att16, tps[:, 0:128])
                    tok = slice(i_abs * 128, (i_abs + 1) * 128)
                    nc.gpsimd.tensor_copy(attn4[:, t, 0, tok], att16)
                    nc.gpsimd.tensor_sub(
                        attn4[:, t, 1, tok], att16, attn4[:, t, 0, tok]
                    )

                def drain(t, jprev, s_ps, I=I):
                    m = jprev - 4 * I
                    q_off = 128 * m if m > 0 else 0
                    pt = ptpool.tile([128, 512], BF16, tag="pt")
                    nc.scalar.activation(
                        pt[:, q_off:512], s_ps[:, q_off:512],
                        mybir.ActivationFunctionType.Exp,
                        scale=SCALE,
                    )
                    o_ps = state[t]["o_ps"]
                    for il in range(4):
                        i_abs = 4 * I + il
                        if jprev <= i_abs:
                            nc.tensor.matmul(
                                o_ps[il][:, 0:129],
                                pt[:, il * 128:(il + 1) * 128],
                                vA[:, jprev, 0:129],
                                start=(jprev == 0),
                                stop=(jprev == i_abs),
                            )
                            if jprev == i_abs:
                                finalize_il(t, il)

                prev = None
                for t in range(QH):
                    state[t] = start_head(t)
                    bucket = pending[t]
                    bucket_total = len(bucket)
                    bucket_done = 0
                    for j in range(njt):
                        mj = j - 4 * I
                        q_off = 128 * mj if mj > 0 else 0
                        s_ps = pwork.tile([128, 512], F32, tag="work")
                        diag = mj >= 0
                        nc.tensor.matmul(
                            s_ps[:, q_off:512],
                            kT[:, j * 128:(j + 1) * 128],
                            qT[:, t, I * 512 + q_off:(I + 1) * 512],
                            start=True, stop=not diag,
                        )
                        if diag:
                            # -60000 causal bias onto the diagonal block
                            nc.tensor.matmul(
                                s_ps[:, q_off:q_off + 128],
                                ident16, maskb,
                                start=False, stop=True,
                                skip_group_check=True,
                            )
                        if prev is not None:
                            tp_, jp_, sp_ = prev
                            drain(tp_, jp_, sp_)
                            target = (j + 1) * bucket_total // njt
                            while bucket and bucket_done < target:
                                bucket.pop(0)()
                                bucket_done += 1
                        elif bucket:
                            # prime the PE while the first exp runs
                            bucket.pop(0)()
                            bucket_done += 1
                        prev = (t, j, s_ps)
                    for fn in bucket:
                        fn()
                tp_, jp_, sp_ = prev
                drain(tp_, jp_, sp_)
                last = I == NI - 1
                units = [make_oproj_unit(I, il, qtr, split_dma=last,
                                          act_evict=last or I <= 1)
                         for il in range(4) for qtr in range(4)]
                pending = [units[0:4], units[4:8], units[8:12], units[12:16]]

            for bucket in pending:
                for fn in bucket:
                    fn()
    nc.finalize()
    return nc


def _hilo(x, scale):
    """Split x*scale into fp8e4 (hi, lo) pieces."""
    import ml_dtypes

    xs = (x * scale).astype(np.float32)
    hi = xs.astype(ml_dtypes.float8_e4m3)
    lo = (xs - hi.astype(np.float32)).astype(ml_dtypes.float8_e4m3)
    return hi, lo


def _pack4(xT, scale, nd, ncols, lhs):
    """Pack xT [nd*128, ncols] into [128, nd, 2, ncols] fp8 pieces.
    lhs=True -> pieces (hi, lo); False -> (lo, hi)."""
    import ml_dtypes

    hi, lo = _hilo(xT, scale)
    outp = np.empty((128, nd, 2, ncols), dtype=ml_dtypes.float8_e4m3)
    h3 = hi.reshape(nd, 128, ncols)
    l3 = lo.reshape(nd, 128, ncols)
    if lhs:
        outp[:, :, 0, :] = h3.transpose(1, 0, 2)
        outp[:, :, 1, :] = l3.transpose(1, 0, 2)
    else:
        outp[:, :, 0, :] = l3.transpose(1, 0, 2)
        outp[:, :, 1, :] = h3.transpose(1, 0, 2)
    return outp


def _pack_inputs(h, position_ids, wq, wk, wv, wo):
    """Host-side shard + transpose + fp8 hi/lo split. Per-core input maps."""
    import ml_dtypes

    h4 = _pack4(np.ascontiguousarray(h.T), SH, ND, S, lhs=False)

    # RoPE tables in transposed orientation, halves duplicated / sign-folded.
    inv = 1.0 / (1e6 ** (np.arange(0, HD, 2, dtype=np.float64) / HD))
    fr = position_ids.astype(np.float64)[None, :] * inv[:, None]   # [64, S]
    cosT = np.cos(fr).astype(np.float16)
    sinT = np.sin(fr).astype(np.float16)
    cosd = np.concatenate([cosT, cosT], axis=0)                    # [128, S]
    sind = np.concatenate([-sinT, sinT], axis=0)
    iden16 = np.eye(128, dtype=np.float16)
    idenbf = np.eye(128).astype(ml_dtypes.bfloat16)
    p_i = np.arange(128)[:, None]
    f_i = np.arange(128)[None, :]
    maskd = np.where(f_i - p_i >= 0, 0.0, MASK_NEG).astype(np.float16)

    in_maps = []
    for c in range(NCORES):
        wq_c = wq[c * 512:(c + 1) * 512, :]          # [512, HID]
        wk_c = wk[c * 128:(c + 1) * 128, :]
        wv_c = wv[c * 128:(c + 1) * 128, :]
        wo_c = wo[:, c * 512:(c + 1) * 512]          # [HID, 512]
        woT = np.ascontiguousarray(wo_c.T)           # [512, HID]
        wqT = np.ascontiguousarray(wq_c.T)           # [HID, 512]
        wq4 = np.stack(
            [_pack4(np.ascontiguousarray(wqT[:, t * 128:(t + 1) * 128]),
                    SW, ND, 128, lhs=True) for t in range(QH)],
            axis=1,
        )                                            # [128, QH, ND, 2, 128]
        in_maps.append({
            "h4": h4,
            "wq4": wq4,
            "wk4": _pack4(np.ascontiguousarray(wk_c.T), SW, ND, 128, lhs=True),
            "wv4": _pack4(np.ascontiguousarray(wv_c.T), SW, ND, 128, lhs=True),
            "wo4": _pack4(woT, SW, QH, HID, lhs=False),
            "cosd": cosd,
            "sind": sind,
            "identd": iden16,
            "identb": idenbf,
            "maskd": maskd,
        })
    return in_maps


def kernel(h, position_ids, wq, wk, wv, wo):
    global LAST_RESULTS
    from concourse.bass_utils import run_bass_kernel_spmd

    if "nc" not in _CACHE:
        _CACHE["nc"] = _build_program()
    nc = _CACHE["nc"]

    in_maps = _pack_inputs(
        np.asarray(h, dtype=np.float32),
        np.asarray(position_ids),
        np.asarray(wq, dtype=np.float32),
        np.asarray(wk, dtype=np.float32),
        np.asarray(wv, dtype=np.float32),
        np.asarray(wo, dtype=np.float32),
    )

    trace = bool(int(os.environ.get("KERNEL_TRACE", "0")))
    res = run_bass_kernel_spmd(
        nc, in_maps, core_ids=list(range(NCORES)), trace=trace
    )
    LAST_RESULTS = res

    acc = np.zeros((S, HID), dtype=np.float32)
    for r in res.results:
        acc += r["out"].astype(np.float32)
    return acc


# revision 27
# speedup vs baseline: 1.0347x; 1.0127x over previous
"""Mixtral GQA attention block (B=1, S=2048, HID=4096, NH=32, NKV=8, HD=128),
8-way tensor-parallel over heads on trn2: each core owns 4 query heads + 1 KV
head (one GQA group), computes its partial output projection, host sums the
8 partials.

Device layout notes:
  - All matmul operands are staged transposed (contraction dim on partitions).
    Host pre-packs every tensor partition-major so DMAs are identity copies.
  - The Q/K/V and output projections run as fp8e4 DoubleRow matmuls with a
    3-term hi/lo error compensation: x@w ~= xh@wh + xh@wl + xl@wh. Pieces are
    packed [128, ktile, piece, N] with lhsT pieces (hi, lo) and rhs pieces
    (lo, hi); per ktile pair one DR instr covers both hi@hi products and one
    DR instr per ktile covers both cross products -> 0.75x the f16 PE cost
    at ~1e-3 accuracy. Tensors are pre-scaled into fp8's normal range
    (h x32, weights x1024, attn x16) and the product descale is folded into
    the PSUM eviction ops.
  - Scores are computed transposed (S^T[k,q]) in f16 so the exp'd tiles
    directly serve as lhsT for the P@V matmul; softmax denominator comes from
    an appended ones-column on V; no max-subtraction. Causal masking of the
    diagonal-band blocks is a -60000 bias added INTO the scores PSUM group
    via an identity-lhsT matmul, so exp needs no follow-up mask multiply.
  - RoPE is applied in the transposed orientation; the half-swap crosses
    partitions and runs as two partition-offset SBUF->SBUF DMAs.
  - The attention drain loop is Activation-bound (exp), so independent PE
    work is software-pipelined into it through a pending-work queue:
    chunk 0 streams the q1..q3 projections through attention(0); chunks
    1..3 stream the previous chunk's output projection. Chunk 0's k/v/q0
    projections are 3-way interleaved per ktile pair so the PE tracks the
    cold-start h DMA stream; DMAs are ordered by first use.
"""

import math
import os
import sys

import numpy as np

sys.path.insert(0, "/opt/trn_rl_repo")

import concourse.bass as bass
import concourse.tile as tile
from concourse import bacc
from concourse import mybir

S = 2048
HID = 4096
NH, NKV, HD = 32, 8, 128
NCORES = 8
QH = NH // NCORES      # 4 query heads per core
ND = HID // 128        # 32 contraction chunks
NI = S // 512          # 4 q-chunks of 512
NJ = S // 128          # 16 k-tiles of 128
SCALE = 1.0 / math.sqrt(HD)

F16 = mybir.dt.float16
BF16 = mybir.dt.bfloat16
F32 = mybir.dt.float32
F8 = mybir.dt.float8e4
DR = mybir.MatmulPerfMode.DoubleRow

SH = 32.0        # fp8 pre-scale on h
SW = 1024.0      # fp8 pre-scale on wq/wk/wv/wo
SA = 16.0        # fp8 pre-scale on attention output
DESCALE_QKV = float(1.0 / (SH * SW))
DESCALE_O = float(1.0 / (SA * SW))
MASK_NEG = -60000.0

_CACHE = {}
LAST_RESULTS = None


def _dr_ops(w_ap, h_ap, nd):
    """(lhsT, rhs) DR operand pairs for the 3-term hi/lo product over `nd`
    ktiles, in ktile-streaming order. w pieces are (hi, lo), h (lo, hi)."""
    ops = []
    for a in range(0, nd, 2):
        ops.append((w_ap(a, 0, False), h_ap(a, 1, False)))      # hi@hi pair
        ops.append((w_ap(a, None, True), h_ap(a, None, True)))  # cross a
        ops.append((w_ap(a + 1, None, True), h_ap(a + 1, None, True)))
    return ops


def _build_program():
    nc = bacc.Bacc()

    h4 = nc.declare_dram_parameter("h4", [128, ND, 2, S], F8, isOutput=False)
    wq4 = nc.declare_dram_parameter("wq4", [128, QH, ND, 2, 128], F8,
                                    isOutput=False)
    wk4 = nc.declare_dram_parameter("wk4", [128, ND, 2, 128], F8, isOutput=False)
    wv4 = nc.declare_dram_parameter("wv4", [128, ND, 2, 128], F8, isOutput=False)
    wo4 = nc.declare_dram_parameter("wo4", [128, QH, 2, HID], F8, isOutput=False)
    cosd = nc.declare_dram_parameter("cosd", [128, S], F16, isOutput=False)
    identd = nc.declare_dram_parameter("identd", [128, 128], F16, isOutput=False)
    identb = nc.declare_dram_parameter("identb", [128, 128], BF16, isOutput=False)
    maskd = nc.declare_dram_parameter("maskd", [128, 128], F16, isOutput=False)
    sind = nc.declare_dram_parameter("sind", [128, S], F16, isOutput=False)
    out = nc.declare_dram_parameter("out", [S, HID], F16, isOutput=True)

    with tile.TileContext(nc) as tc:
        with (
            tc.tile_pool(name="consts", bufs=1) as consts,
            tc.tile_pool(name="hpool", bufs=10) as hpool,
            tc.tile_pool(name="ptpool", bufs=5) as ptpool,
            tc.tile_pool(name="rtmp", bufs=3) as rtmp,
            tc.tile_pool(name="small", bufs=8) as small,
            tc.tile_pool(name="orow", bufs=5) as orowp,
            tc.tile_pool(name="pproj", bufs=2, space="PSUM") as pproj,
            tc.tile_pool(name="pwork", bufs=2, space="PSUM") as pwork,
            tc.tile_pool(name="popsum", bufs=4, space="PSUM") as popsum,
        ):
            wq_sb = consts.tile([128, QH, ND, 2, 128], F8)
            cos_sb = consts.tile([128, S], F16)
            sin_sb = consts.tile([128, S], F16)
            wk_sb = consts.tile([128, ND, 2, 128], F8)
            wv_sb = consts.tile([128, ND, 2, 128], F8)
            wo_sb = consts.tile([128, QH, 2, HID], F8)
            ident16 = consts.tile([128, 128], F16)
            identbf = consts.tile([128, 128], BF16)
            maskb = consts.tile([128, 128], F16)

            hts0 = []
            for dq in range(8):
                t_h = hpool.tile([128, 4, 2, 512], F8, tag="ht",
                                 name=f"ht_0_{dq}")
                hts0.append(t_h)

            def ht_dma(tiles, I, dq):
                nc.sync.dma_start(
                    out=tiles[dq],
                    in_=h4[:, dq * 4:(dq + 1) * 4, :, I * 512:(I + 1) * 512],
                )

            def wq_dma(t, half):
                dsl = slice(half * 16, (half + 1) * 16)
                nc.sync.dma_start(out=wq_sb[:, t, dsl, :, :],
                                  in_=wq4[:, t, dsl, :, :])

            # Startup DMA order = first-use order: k/v/q0 weights lead the
            # cold h stream; tables next; q1..q3 heads land during
            # attention(0); wo column blocks land before attention(1).
            nc.sync.dma_start(out=wk_sb[:, 0:8, :, :], in_=wk4[:, 0:8, :, :])
            nc.sync.dma_start(out=hts0[0][:, 0:2, :, :],
                              in_=h4[:, 0:2, :, 0:512])
            nc.sync.dma_start(out=wv_sb[:, 0:8, :, :], in_=wv4[:, 0:8, :, :])
            nc.sync.dma_start(out=hts0[0][:, 2:4, :, :],
                              in_=h4[:, 2:4, :, 0:512])
            nc.sync.dma_start(out=wq_sb[:, 0, 0:8, :, :],
                              in_=wq4[:, 0, 0:8, :, :])
            nc.sync.dma_start(out=wk_sb[:, 8:16, :, :], in_=wk4[:, 8:16, :, :])
            nc.sync.dma_start(out=wv_sb[:, 8:16, :, :], in_=wv4[:, 8:16, :, :])
            nc.sync.dma_start(out=wq_sb[:, 0, 8:16, :, :],
                              in_=wq4[:, 0, 8:16, :, :])
            nc.sync.dma_start(out=wk_sb[:, 16:32, :, :], in_=wk4[:, 16:32, :, :])
            ht_dma(hts0, 0, 1)
            nc.sync.dma_start(out=wv_sb[:, 16:32, :, :], in_=wv4[:, 16:32, :, :])
            ht_dma(hts0, 0, 2)
            wq_dma(0, 1)
            ht_dma(hts0, 0, 3)
            for dq in range(4, 8):
                ht_dma(hts0, 0, dq)
            nc.sync.dma_start(out=cos_sb, in_=cosd[:, :])
            nc.sync.dma_start(out=sin_sb, in_=sind[:, :])
            nc.sync.dma_start(out=identbf, in_=identb[:, :])
            nc.sync.dma_start(out=maskb, in_=maskd[:, :])
            nc.sync.dma_start(out=ident16, in_=identd[:, :])
            for t in range(1, QH):
                wq_dma(t, 0)
                wq_dma(t, 1)

            qT = consts.tile([128, QH, S], F16)    # roped q, transposed
            kT = consts.tile([128, S], F16)        # roped k, transposed
            # V' tiles: per k-tile j, [128 tokens, 128 ch + ones column]
            vA = consts.tile([128, NJ, 132], BF16)
            for j in range(NJ):
                nc.vector.memset(vA[:, j, 128:129], 1.0)
            # attn output pieces for the fp8 O-projection: [hd, head, piece, S]
            attn4 = consts.tile([128, QH, 2, S], F8)

            def w_ap_of(w_tile, t=None):
                if t is None:
                    def w_ap(a, piece, single):
                        if single:
                            return w_tile[:, a, 0:2, :]
                        return w_tile[:, a:a + 2, piece, :]
                else:
                    def w_ap(a, piece, single):
                        if single:
                            return w_tile[:, t, a, 0:2, :]
                        return w_tile[:, t, a:a + 2, piece, :]
                return w_ap

            def h_ap_of(hts):
                def h_ap(a, piece, single):
                    if single:
                        return hts[a // 4][:, a % 4, 0:2, :]
                    return hts[a // 4][:, a % 4:a % 4 + 2, piece, :]
                return h_ap

            def emit_chain(ps, ops, lo, hi_i):
                for i in range(lo, hi_i):
                    nc.tensor.matmul(
                        ps, ops[i][0], ops[i][1],
                        start=(i == 0), stop=(i == len(ops) - 1),
                        perf_mode=DR,
                    )

            def rope_into(ps, dst, nsl):
                """ps: PSUM [128, 512] f32 pre-rope (transposed layout,
                scaled by SH*SW). dst: f16 SBUF slice. The half-swap
                crosses partitions -> two partition-offset DMAs."""
                cpy = rtmp.tile([128, 512], F16, tag="ropecpy")
                nc.scalar.mul(cpy, ps, DESCALE_QKV)
                sw = rtmp.tile([128, 512], F16, tag="ropesw")
                nc.sync.dma_start(out=sw[0:64, :], in_=cpy[64:128, :])
                nc.sync.dma_start(out=sw[64:128, :], in_=cpy[0:64, :])
                tmp2 = rtmp.tile([128, 512], F16, tag="ropecos")
                nc.gpsimd.tensor_mul(tmp2, cpy, cos_sb[:, nsl])
                nc.gpsimd.tensor_mul(sw, sw, sin_sb[:, nsl])
                nc.vector.tensor_add(dst, tmp2, sw)

            def v_stage(v_ps, I):
                vt_sb = small.tile([128, 512], BF16, tag="vt")
                nc.vector.tensor_scalar_mul(vt_sb, v_ps, DESCALE_QKV)
                for jj in range(4):
                    tps = pwork.tile([128, 128], BF16, tag="work")
                    nc.tensor.transpose(
                        tps, vt_sb[:, jj * 128:(jj + 1) * 128], identbf
                    )
                    nc.vector.tensor_copy(vA[:, 4 * I + jj, 0:128], tps)

            # ---- O-projection emission units ------------------------------
            def make_oproj_unit(I, il, qtr, split_dma=False, act_evict=False,
                                use_pwork=False):
                i_abs = 4 * I + il
                tok = slice(i_abs * 128, (i_abs + 1) * 128)

                def a_ap(a, piece, single):
                    if single:
                        return attn4[:, a, 0:2, tok]
                    return attn4[:, a:a + 2, piece, tok]

                def emit():
                    orow = orowp.tile([128, 1024], F16, tag="orow")
                    for mc in range(2):
                        mq = qtr * 1024 + mc * 512

                        def w_ap(a, piece, single, mq=mq):
                            if single:
                                return wo_sb[:, a, 0:2, mq:mq + 512]
                            return wo_sb[:, a:a + 2, piece, mq:mq + 512]

                        pool = pwork if use_pwork else pproj
                        ptag = "work" if use_pwork else "proj"
                        op_ps = pool.tile([128, 512], F32, tag=ptag)
                        ops = _dr_ops(a_ap, w_ap, QH)
                        emit_chain(op_ps, ops, 0, len(ops))
                        osl = orow[:, mc * 512:(mc + 1) * 512]
                        if act_evict and mc == 1:
                            nc.scalar.mul(osl, op_ps, DESCALE_O)
                        else:
                            nc.vector.tensor_scalar_mul(osl, op_ps, DESCALE_O)
                        if split_dma:
                            nc.sync.dma_start(
                                out=out[i_abs * 128:(i_abs + 1) * 128,
                                        mq:mq + 512],
                                in_=osl,
                            )
                    if not split_dma:
                        nc.sync.dma_start(
                            out=out[i_abs * 128:(i_abs + 1) * 128,
                                    qtr * 1024:(qtr + 1) * 1024],
                            in_=orow,
                        )
                return emit

            # ---- chunk 0 cold start: 3-way interleaved k/v/q0 -------------
            nsl0 = slice(0, 512)
            hap0 = h_ap_of(hts0)
            k_ps = pproj.tile([128, 512], F32, tag="proj")
            v_ps = pproj.tile([128, 512], F32, tag="proj")
            q0_ps = popsum.tile([128, 512], F32, tag="opsum", name="q_ps_0_0")
            k_ops = _dr_ops(w_ap_of(wk_sb), hap0, ND)
            v_ops = _dr_ops(w_ap_of(wv_sb), hap0, ND)
            q0_ops = _dr_ops(w_ap_of(wq_sb, 0), hap0, ND)
            for pr in range(ND // 2):
                emit_chain(k_ps, k_ops, 3 * pr, 3 * pr + 3)
                emit_chain(v_ps, v_ops, 3 * pr, 3 * pr + 3)
                emit_chain(q0_ps, q0_ops, 3 * pr, 3 * pr + 3)
            rope_into(k_ps, kT[:, nsl0], nsl0)
            v_stage(v_ps, 0)
            rope_into(q0_ps, qT[:, 0, nsl0], nsl0)

            # chunk-0 pending work: q1..q3 projections streamed through
            # attention(0)'s drains. The chunk-1 h prefetch is emitted right
            # after the LAST q3 group reading each chunk-0 sub-tile (the
            # hpool buffer reuse must come after every reader is emitted),
            # and the wo column blocks trail at the end.
            pending = []
            q_state = {}
            hts_next = []

            def make_q_group(t, pr, hts, nsl):
                def emit():
                    if t not in q_state:
                        q_state[t] = {
                            "ps": pproj.tile([128, 512], F32, tag="proj",
                                             name=f"q_ps_0_{t}"),
                            "ops": _dr_ops(w_ap_of(wq_sb, t), h_ap_of(hts),
                                           ND),
                        }
                    st = q_state[t]
                    emit_chain(st["ps"], st["ops"], 3 * pr, 3 * pr + 3)
                    if pr == ND // 2 - 1:
                        rope_into(st["ps"], qT[:, t, nsl], nsl)
                        del q_state[t]
                return emit

            def make_h1_prefetch(dq):
                def emit():
                    t_h = hpool.tile([128, 4, 2, 512], F8, tag="ht",
                                     name=f"ht_1_{dq}")
                    assert len(hts_next) == dq
                    hts_next.append(t_h)
                    ht_dma(hts_next, 1, dq)
                return emit

            def make_wo_dma(cb):
                def emit():
                    csl = slice(cb * 1024, (cb + 1) * 1024)
                    nc.sync.dma_start(out=wo_sb[:, :, :, csl],
                                      in_=wo4[:, :, :, csl])
                return emit

            # per-head buckets: bucket t is fully emitted during head t's
            # drains, so head t+1's scores (which need qT[t+1]) are never
            # enqueued ahead of the projection work they depend on.
            pending = [[], [], [], []]
            for t in range(1, QH):
                for pr in range(ND // 2):
                    pending[t - 1].append(make_q_group(t, pr, hts0, nsl0))
                    if t == QH - 1 and pr % 2 == 1:
                        pending[t - 1].append(make_h1_prefetch(pr // 2))
            for cb in range(4):
                pending[3].append(make_wo_dma(cb))

            hts_cur = hts_next
            for I in range(NI):
                nsl = slice(I * 512, (I + 1) * 512)
                hts = hts_cur

                if I > 0:
                    # ---- projections (h prefetched; PE-bound) ------------
                    k_ps = pproj.tile([128, 512], F32, tag="proj")
                    emit_chain(k_ps, _dr_ops(w_ap_of(wk_sb), h_ap_of(hts),
                                             ND), 0, 3 * (ND // 2))
                    rope_into(k_ps, kT[:, nsl], nsl)

                    v_ps = pproj.tile([128, 512], F32, tag="proj")
                    emit_chain(v_ps, _dr_ops(w_ap_of(wv_sb), h_ap_of(hts),
                                             ND), 0, 3 * (ND // 2))
                    v_stage(v_ps, I)

                    for t in range(QH):
                        q_ps = pproj.tile([128, 512], F32, tag="proj")
                        emit_chain(q_ps, _dr_ops(w_ap_of(wq_sb, t),
                                                 h_ap_of(hts), ND),
                                   0, 3 * (ND // 2))
                        rope_into(q_ps, qT[:, t, nsl], nsl)

                    # prefetch next pass's h slice
                    if I + 1 < NI:
                        hts_cur = []
                        for dq in range(8):
                            t_h = hpool.tile([128, 4, 2, 512], F8, tag="ht",
                                             name=f"ht_{I + 1}_{dq}")
                            hts_cur.append(t_h)
                            ht_dma(hts_cur, I + 1, dq)

                # ---- attention for q-chunk I, with pending PE work
                # interleaved into the drain loop ----------------------
                njt = 4 * I + 4   # k-tiles in causal range of this chunk
                state = {}

                def start_head(t, I=I):
                    o_ps = [
                        popsum.tile([128, 132], F32, tag="opsum",
                                    name=f"o_ps_{I}_{t}_{il}")
                        for il in range(4)
                    ]
                    return {"o_ps": o_ps}

                def finalize_il(t, il, I=I):
                    o_ps = state[t]["o_ps"]
                    i_abs = 4 * I + il
                    recip = small.tile([128, 1], F32, tag="recip")
                    nc.vector.reciprocal(recip, o_ps[il][:, 128:129])
                    # osc carries the fp8 pre-scale SA so the casts are
                    # plain copy/sub ops that can run on the Pool engine
                    osc = small.tile([128, 128], F16, tag="osc")
                    nc.vector.tensor_scalar(
                        osc, o_ps[il][:, 0:128], recip, SA,
                        mybir.AluOpType.mult, mybir.AluOpType.mult,
                    )
                    tps = popsum.tile([128, 132], F16, tag="opsum",
                                      name=f"tps_{I}_{t}_{il}")
                    nc.tensor.transpose(tps[:, 0:128], osc, ident16)
                    att16 = small.tile([128, 128], F16, tag="att16")
                    nc.vector.tensor_copy(att16, tps[:, 0:128])
                    tok = slice(i_abs * 128, (i_abs + 1) * 128)
                    nc.gpsimd.tensor_copy(attn4[:, t, 0, tok], att16)
                    nc.gpsimd.tensor_sub(
                        attn4[:, t, 1, tok], att16, attn4[:, t, 0, tok]
                    )

                def drain(t, jprev, s_ps, I=I):
                    m = jprev - 4 * I
                    q_off = 128 * m if m > 0 else 0
                    pt = ptpool.tile([128, 512], BF16, tag="pt")
                    nc.scalar.activation(
                        pt[:, q_off:512], s_ps[:, q_off:512],
                        mybir.ActivationFunctionType.Exp,
                        scale=SCALE,
                    )
                    o_ps = state[t]["o_ps"]
                    for il in range(4):
                        i_abs = 4 * I + il
                        if jprev <= i_abs:
                            nc.tensor.matmul(
                                o_ps[il][:, 0:129],
                                pt[:, il * 128:(il + 1) * 128],
                                vA[:, jprev, 0:129],
                                start=(jprev == 0),
                                stop=(jprev == i_abs),
                            )
                            if jprev == i_abs:
                                finalize_il(t, il)

                prev = None
                for t in range(QH):
                    state[t] = start_head(t)
                    bucket = pending[t]
                    bucket_total = len(bucket)
                    bucket_done = 0
                    for j in range(njt):
                        mj = j - 4 * I
                        q_off = 128 * mj if mj > 0 else 0
                        s_ps = pwork.tile([128, 512], F32, tag="work")
                        diag = mj >= 0
                        nc.tensor.matmul(
                            s_ps[:, q_off:512],
                            kT[:, j * 128:(j + 1) * 128],
                            qT[:, t, I * 512 + q_off:(I + 1) * 512],
                            start=True, stop=not diag,
                        )
                        if diag:
                            # -60000 causal bias onto the diagonal block
                            nc.tensor.matmul(
                                s_ps[:, q_off:q_off + 128],
                                ident16, maskb,
                                start=False, stop=True,
                                skip_group_check=True,
                            )
                        if prev is not None:
                            tp_, jp_, sp_ = prev
                            drain(tp_, jp_, sp_)
                            target = (j + 1) * bucket_total // njt
                            while bucket and bucket_done < target:
                                bucket.pop(0)()
                                bucket_done += 1
                        elif bucket:
                            # prime the PE while the first exp runs
                            bucket.pop(0)()
                            bucket_done += 1
                        prev = (t, j, s_ps)
                    for fn in bucket:
                        fn()
                tp_, jp_, sp_ = prev
                drain(tp_, jp_, sp_)
                last = I == NI - 1
                units = [make_oproj_unit(I, il, qtr, split_dma=last,
                                          act_evict=last or I <= 1,
                                          use_pwork=last and (il + qtr) % 2 == 1)
                         for il in range(4) for qtr in range(4)]
                pending = [units[0:4], units[4:8], units[8:12], units[12:16]]

            for bucket in pending:
                for fn in bucket:
                    fn()
    nc.finalize()
    return nc


def _hilo(x, scale):
    """Split x*scale into fp8e4 (hi, lo) pieces."""
    import ml_dtypes

    xs = (x * scale).astype(np.float32)
    hi = xs.astype(ml_dtypes.float8_e4m3)
    lo = (xs - hi.astype(np.float32)).astype(ml_dtypes.float8_e4m3)
    return hi, lo


def _pack4(xT, scale, nd, ncols, lhs):
    """Pack xT [nd*128, ncols] into [128, nd, 2, ncols] fp8 pieces.
    lhs=True -> pieces (hi, lo); False -> (lo, hi)."""
    import ml_dtypes

    hi, lo = _hilo(xT, scale)
    outp = np.empty((128, nd, 2, ncols), dtype=ml_dtypes.float8_e4m3)
    h3 = hi.reshape(nd, 128, ncols)
    l3 = lo.reshape(nd, 128, ncols)
    if lhs:
        outp[:, :, 0, :] = h3.transpose(1, 0, 2)
        outp[:, :, 1, :] = l3.transpose(1, 0, 2)
    else:
        outp[:, :, 0, :] = l3.transpose(1, 0, 2)
        outp[:, :, 1, :] = h3.transpose(1, 0, 2)
    return outp


def _pack_inputs(h, position_ids, wq, wk, wv, wo):
    """Host-side shard + transpose + fp8 hi/lo split. Per-core input maps."""
    import ml_dtypes

    h4 = _pack4(np.ascontiguousarray(h.T), SH, ND, S, lhs=False)

    # RoPE tables in transposed orientation, halves duplicated / sign-folded.
    inv = 1.0 / (1e6 ** (np.arange(0, HD, 2, dtype=np.float64) / HD))
    fr = position_ids.astype(np.float64)[None, :] * inv[:, None]   # [64, S]
    cosT = np.cos(fr).astype(np.float16)
    sinT = np.sin(fr).astype(np.float16)
    cosd = np.concatenate([cosT, cosT], axis=0)                    # [128, S]
    sind = np.concatenate([-sinT, sinT], axis=0)
    iden16 = np.eye(128, dtype=np.float16)
    idenbf = np.eye(128).astype(ml_dtypes.bfloat16)
    p_i = np.arange(128)[:, None]
    f_i = np.arange(128)[None, :]
    maskd = np.where(f_i - p_i >= 0, 0.0, MASK_NEG).astype(np.float16)

    in_maps = []
    for c in range(NCORES):
        wq_c = wq[c * 512:(c + 1) * 512, :]          # [512, HID]
        wk_c = wk[c * 128:(c + 1) * 128, :]
        wv_c = wv[c * 128:(c + 1) * 128, :]
        wo_c = wo[:, c * 512:(c + 1) * 512]          # [HID, 512]
        woT = np.ascontiguousarray(wo_c.T)           # [512, HID]
        wqT = np.ascontiguousarray(wq_c.T)           # [HID, 512]
        wq4 = np.stack(
            [_pack4(np.ascontiguousarray(wqT[:, t * 128:(t + 1) * 128]),
                    SW, ND, 128, lhs=True) for t in range(QH)],
            axis=1,
        )                                            # [128, QH, ND, 2, 128]
        in_maps.append({
            "h4": h4,
            "wq4": wq4,
            "wk4": _pack4(np.ascontiguousarray(wk_c.T), SW, ND, 128, lhs=True),
            "wv4": _pack4(np.ascontiguousarray(wv_c.T), SW, ND, 128, lhs=True),
            "wo4": _pack4(woT, SW, QH, HID, lhs=False),
            "cosd": cosd,
            "sind": sind,
            "identd": iden16,
            "identb": idenbf,
            "maskd": maskd,
        })
    return in_maps


def kernel(h, position_ids, wq, wk, wv, wo):
    global LAST_RESULTS
    from concourse.bass_utils import run_bass_kernel_spmd

    if "nc" not in _CACHE:
        _CACHE["nc"] = _build_program()
    nc = _CACHE["nc"]

    in_maps = _pack_inputs(
        np.asarray(h, dtype=np.float32),
        np.asarray(position_ids),
        np.asarray(wq, dtype=np.float32),
        np.asarray(wk, dtype=np.float32),
        np.asarray(wv, dtype=np.float32),
        np.asarray(wo, dtype=np.float32),
    )

    trace = bool(int(os.environ.get("KERNEL_TRACE", "0")))
    res = run_bass_kernel_spmd(
        nc, in_maps, core_ids=list(range(NCORES)), trace=trace
    )
    LAST_RESULTS = res

    acc = np.zeros((S, HID), dtype=np.float32)
    for r in res.results:
        acc += r["out"].astype(np.float32)
    return acc


# revision 28
# speedup vs baseline: 1.0358x; 1.0011x over previous
"""Mixtral GQA attention block (B=1, S=2048, HID=4096, NH=32, NKV=8, HD=128),
8-way tensor-parallel over heads on trn2: each core owns 4 query heads + 1 KV
head (one GQA group), computes its partial output projection, host sums the
8 partials.

Device layout notes:
  - All matmul operands are staged transposed (contraction dim on partitions).
    Host pre-packs every tensor partition-major so DMAs are identity copies.
  - The Q/K/V and output projections run as fp8e4 DoubleRow matmuls with a
    3-term hi/lo error compensation: x@w ~= xh@wh + xh@wl + xl@wh. Pieces are
    packed [128, ktile, piece, N] with lhsT pieces (hi, lo) and rhs pieces
    (lo, hi); per ktile pair one DR instr covers both hi@hi products and one
    DR instr per ktile covers both cross products -> 0.75x the f16 PE cost
    at ~1e-3 accuracy. Tensors are pre-scaled into fp8's normal range
    (h x32, weights x1024, attn x16) and the product descale is folded into
    the PSUM eviction ops.
  - Scores are computed transposed (S^T[k,q]) in f16 so the exp'd tiles
    directly serve as lhsT for the P@V matmul; softmax denominator comes from
    an appended ones-column on V; no max-subtraction. Causal masking of the
    diagonal-band blocks is a -60000 bias added INTO the scores PSUM group
    via an identity-lhsT matmul, so exp needs no follow-up mask multiply.
  - RoPE is applied in the transposed orientation; the half-swap crosses
    partitions and runs as two partition-offset SBUF->SBUF DMAs.
  - The attention drain loop is Activation-bound (exp), so independent PE
    work is software-pipelined into it through a pending-work queue:
    chunk 0 streams the q1..q3 projections through attention(0); chunks
    1..3 stream the previous chunk's output projection. Chunk 0's k/v/q0
    projections are 3-way interleaved per ktile pair so the PE tracks the
    cold-start h DMA stream; DMAs are ordered by first use.
"""

import math
import os
import sys

import numpy as np

sys.path.insert(0, "/opt/trn_rl_repo")

import concourse.bass as bass
import concourse.tile as tile
from concourse import bacc
from concourse import mybir

S = 2048
HID = 4096
NH, NKV, HD = 32, 8, 128
NCORES = 8
QH = NH // NCORES      # 4 query heads per core
ND = HID // 128        # 32 contraction chunks
NI = S // 512          # 4 q-chunks of 512
NJ = S // 128          # 16 k-tiles of 128
SCALE = 1.0 / math.sqrt(HD)

F16 = mybir.dt.float16
BF16 = mybir.dt.bfloat16
F32 = mybir.dt.float32
F8 = mybir.dt.float8e4
DR = mybir.MatmulPerfMode.DoubleRow

SH = 32.0        # fp8 pre-scale on h
SW = 1024.0      # fp8 pre-scale on wq/wk/wv/wo
SA = 16.0        # fp8 pre-scale on attention output
DESCALE_QKV = float(1.0 / (SH * SW))
DESCALE_O = float(1.0 / (SA * SW))
MASK_NEG = -60000.0

_CACHE = {}
LAST_RESULTS = None


def _dr_ops(w_ap, h_ap, nd):
    """(lhsT, rhs) DR operand pairs for the 3-term hi/lo product over `nd`
    ktiles, in ktile-streaming order. w pieces are (hi, lo), h (lo, hi)."""
    ops = []
    for a in range(0, nd, 2):
        ops.append((w_ap(a, 0, False), h_ap(a, 1, False)))      # hi@hi pair
        ops.append((w_ap(a, None, True), h_ap(a, None, True)))  # cross a
        ops.append((w_ap(a + 1, None, True), h_ap(a + 1, None, True)))
    return ops


def _build_program():
    nc = bacc.Bacc()

    h4 = nc.declare_dram_parameter("h4", [128, ND, 2, S], F8, isOutput=False)
    wq4 = nc.declare_dram_parameter("wq4", [128, QH, ND, 2, 128], F8,
                                    isOutput=False)
    wk4 = nc.declare_dram_parameter("wk4", [128, ND, 2, 128], F8, isOutput=False)
    wv4 = nc.declare_dram_parameter("wv4", [128, ND, 2, 128], F8, isOutput=False)
    wo4 = nc.declare_dram_parameter("wo4", [128, QH, 2, HID], F8, isOutput=False)
    cosd = nc.declare_dram_parameter("cosd", [128, S], F16, isOutput=False)
    identd = nc.declare_dram_parameter("identd", [128, 128], F16, isOutput=False)
    identb = nc.declare_dram_parameter("identb", [128, 128], BF16, isOutput=False)
    maskd = nc.declare_dram_parameter("maskd", [128, 128], F16, isOutput=False)
    sind = nc.declare_dram_parameter("sind", [128, S], F16, isOutput=False)
    out = nc.declare_dram_parameter("out", [S, HID], F16, isOutput=True)

    with tile.TileContext(nc) as tc:
        with (
            tc.tile_pool(name="consts", bufs=1) as consts,
            tc.tile_pool(name="hpool", bufs=10) as hpool,
            tc.tile_pool(name="ptpool", bufs=5) as ptpool,
            tc.tile_pool(name="rtmp", bufs=3) as rtmp,
            tc.tile_pool(name="small", bufs=8) as small,
            tc.tile_pool(name="orow", bufs=5) as orowp,
            tc.tile_pool(name="pproj", bufs=2, space="PSUM") as pproj,
            tc.tile_pool(name="pwork", bufs=2, space="PSUM") as pwork,
            tc.tile_pool(name="popsum", bufs=4, space="PSUM") as popsum,
        ):
            wq_sb = consts.tile([128, QH, ND, 2, 128], F8)
            cos_sb = consts.tile([128, S], F16)
            sin_sb = consts.tile([128, S], F16)
            wk_sb = consts.tile([128, ND, 2, 128], F8)
            wv_sb = consts.tile([128, ND, 2, 128], F8)
            wo_sb = consts.tile([128, QH, 2, HID], F8)
            ident16 = consts.tile([128, 128], F16)
            identbf = consts.tile([128, 128], BF16)
            maskb = consts.tile([128, 128], F16)

            hts0 = []
            for dq in range(8):
                t_h = hpool.tile([128, 4, 2, 512], F8, tag="ht",
                                 name=f"ht_0_{dq}")
                hts0.append(t_h)

            def ht_dma(tiles, I, dq):
                nc.sync.dma_start(
                    out=tiles[dq],
                    in_=h4[:, dq * 4:(dq + 1) * 4, :, I * 512:(I + 1) * 512],
                )

            def wq_dma(t, half):
                dsl = slice(half * 16, (half + 1) * 16)
                nc.sync.dma_start(out=wq_sb[:, t, dsl, :, :],
                                  in_=wq4[:, t, dsl, :, :])

            # Startup DMA order = first-use order: k/v/q0 weights lead the
            # cold h stream; tables next; q1..q3 heads land during
            # attention(0); wo column blocks land before attention(1).
            nc.sync.dma_start(out=wk_sb[:, 0:8, :, :], in_=wk4[:, 0:8, :, :])
            nc.sync.dma_start(out=hts0[0][:, 0:2, :, :],
                              in_=h4[:, 0:2, :, 0:512])
            nc.sync.dma_start(out=wv_sb[:, 0:8, :, :], in_=wv4[:, 0:8, :, :])
            nc.sync.dma_start(out=hts0[0][:, 2:4, :, :],
                              in_=h4[:, 2:4, :, 0:512])
            nc.sync.dma_start(out=wq_sb[:, 0, 0:8, :, :],
                              in_=wq4[:, 0, 0:8, :, :])
            nc.sync.dma_start(out=wk_sb[:, 8:16, :, :], in_=wk4[:, 8:16, :, :])
            nc.sync.dma_start(out=wv_sb[:, 8:16, :, :], in_=wv4[:, 8:16, :, :])
            nc.sync.dma_start(out=wq_sb[:, 0, 8:16, :, :],
                              in_=wq4[:, 0, 8:16, :, :])
            nc.sync.dma_start(out=wk_sb[:, 16:32, :, :], in_=wk4[:, 16:32, :, :])
            ht_dma(hts0, 0, 1)
            nc.sync.dma_start(out=wv_sb[:, 16:32, :, :], in_=wv4[:, 16:32, :, :])
            ht_dma(hts0, 0, 2)
            wq_dma(0, 1)
            ht_dma(hts0, 0, 3)
            for dq in range(4, 8):
                ht_dma(hts0, 0, dq)
            nc.sync.dma_start(out=cos_sb[:, 0:512], in_=cosd[:, 0:512])
            nc.sync.dma_start(out=sin_sb[:, 0:512], in_=sind[:, 0:512])
            nc.sync.dma_start(out=identbf, in_=identb[:, :])
            nc.sync.dma_start(out=maskb, in_=maskd[:, :])
            nc.sync.dma_start(out=ident16, in_=identd[:, :])
            wq_dma(1, 0)
            wq_dma(1, 1)
            nc.sync.dma_start(out=cos_sb[:, 512:S], in_=cosd[:, 512:S])
            nc.sync.dma_start(out=sin_sb[:, 512:S], in_=sind[:, 512:S])
            for t in range(2, QH):
                wq_dma(t, 0)
                wq_dma(t, 1)

            qT = consts.tile([128, QH, S], F16)    # roped q, transposed
            kT = consts.tile([128, S], F16)        # roped k, transposed
            # V' tiles: per k-tile j, [128 tokens, 128 ch + ones column]
            vA = consts.tile([128, NJ, 132], BF16)
            for j in range(NJ):
                nc.vector.memset(vA[:, j, 128:129], 1.0)
            # attn output pieces for the fp8 O-projection: [hd, head, piece, S]
            attn4 = consts.tile([128, QH, 2, S], F8)

            def w_ap_of(w_tile, t=None):
                if t is None:
                    def w_ap(a, piece, single):
                        if single:
                            return w_tile[:, a, 0:2, :]
                        return w_tile[:, a:a + 2, piece, :]
                else:
                    def w_ap(a, piece, single):
                        if single:
                            return w_tile[:, t, a, 0:2, :]
                        return w_tile[:, t, a:a + 2, piece, :]
                return w_ap

            def h_ap_of(hts):
                def h_ap(a, piece, single):
                    if single:
                        return hts[a // 4][:, a % 4, 0:2, :]
                    return hts[a // 4][:, a % 4:a % 4 + 2, piece, :]
                return h_ap

            def emit_chain(ps, ops, lo, hi_i):
                for i in range(lo, hi_i):
                    nc.tensor.matmul(
                        ps, ops[i][0], ops[i][1],
                        start=(i == 0), stop=(i == len(ops) - 1),
                        perf_mode=DR,
                    )

            def rope_into(ps, dst, nsl):
                """ps: PSUM [128, 512] f32 pre-rope (transposed layout,
                scaled by SH*SW). dst: f16 SBUF slice. The half-swap
                crosses partitions -> two partition-offset DMAs."""
                cpy = rtmp.tile([128, 512], F16, tag="ropecpy")
                nc.scalar.mul(cpy, ps, DESCALE_QKV)
                sw = rtmp.tile([128, 512], F16, tag="ropesw")
                nc.sync.dma_start(out=sw[0:64, :], in_=cpy[64:128, :])
                nc.sync.dma_start(out=sw[64:128, :], in_=cpy[0:64, :])
                tmp2 = rtmp.tile([128, 512], F16, tag="ropecos")
                nc.gpsimd.tensor_mul(tmp2, cpy, cos_sb[:, nsl])
                nc.gpsimd.tensor_mul(sw, sw, sin_sb[:, nsl])
                nc.vector.tensor_add(dst, tmp2, sw)

            def v_stage(v_ps, I):
                vt_sb = small.tile([128, 512], BF16, tag="vt")
                nc.vector.tensor_scalar_mul(vt_sb, v_ps, DESCALE_QKV)
                for jj in range(4):
                    tps = pwork.tile([128, 128], BF16, tag="work")
                    nc.tensor.transpose(
                        tps, vt_sb[:, jj * 128:(jj + 1) * 128], identbf
                    )
                    nc.vector.tensor_copy(vA[:, 4 * I + jj, 0:128], tps)

            # ---- O-projection emission units ------------------------------
            def make_oproj_unit(I, il, qtr, split_dma=False, act_evict=False,
                                use_pwork=False, fine=False):
                i_abs = 4 * I + il
                tok = slice(i_abs * 128, (i_abs + 1) * 128)

                def a_ap(a, piece, single):
                    if single:
                        return attn4[:, a, 0:2, tok]
                    return attn4[:, a:a + 2, piece, tok]

                def emit():
                    orow = orowp.tile([128, 1024], F16, tag="orow")
                    for mc in range(2):
                        mq = qtr * 1024 + mc * 512

                        def w_ap(a, piece, single, mq=mq):
                            if single:
                                return wo_sb[:, a, 0:2, mq:mq + 512]
                            return wo_sb[:, a:a + 2, piece, mq:mq + 512]

                        pool = pwork if use_pwork else pproj
                        ptag = "work" if use_pwork else "proj"
                        op_ps = pool.tile([128, 512], F32, tag=ptag)
                        ops = _dr_ops(a_ap, w_ap, QH)
                        emit_chain(op_ps, ops, 0, len(ops))
                        osl = orow[:, mc * 512:(mc + 1) * 512]
                        if fine and mc == 1:
                            for qq in range(2):
                                qsl = slice(qq * 256, (qq + 1) * 256)
                                eng = nc.scalar.mul if qq else \
                                    nc.vector.tensor_scalar_mul
                                eng(osl[:, qsl], op_ps[:, qsl], DESCALE_O)
                                nc.sync.dma_start(
                                    out=out[i_abs * 128:(i_abs + 1) * 128,
                                            mq + qq * 256:mq + qq * 256 + 256],
                                    in_=osl[:, qsl],
                                )
                            continue
                        if act_evict and mc == 1:
                            nc.scalar.mul(osl, op_ps, DESCALE_O)
                        else:
                            nc.vector.tensor_scalar_mul(osl, op_ps, DESCALE_O)
                        if split_dma:
                            nc.sync.dma_start(
                                out=out[i_abs * 128:(i_abs + 1) * 128,
                                        mq:mq + 512],
                                in_=osl,
                            )
                    if not split_dma:
                        nc.sync.dma_start(
                            out=out[i_abs * 128:(i_abs + 1) * 128,
                                    qtr * 1024:(qtr + 1) * 1024],
                            in_=orow,
                        )
                return emit

            # ---- chunk 0 cold start: 3-way interleaved k/v/q0 -------------
            nsl0 = slice(0, 512)
            hap0 = h_ap_of(hts0)
            k_ps = pproj.tile([128, 512], F32, tag="proj")
            v_ps = pproj.tile([128, 512], F32, tag="proj")
            q0_ps = popsum.tile([128, 512], F32, tag="opsum", name="q_ps_0_0")
            k_ops = _dr_ops(w_ap_of(wk_sb), hap0, ND)
            v_ops = _dr_ops(w_ap_of(wv_sb), hap0, ND)
            q0_ops = _dr_ops(w_ap_of(wq_sb, 0), hap0, ND)
            for pr in range(ND // 2):
                emit_chain(k_ps, k_ops, 3 * pr, 3 * pr + 3)
                emit_chain(v_ps, v_ops, 3 * pr, 3 * pr + 3)
                emit_chain(q0_ps, q0_ops, 3 * pr, 3 * pr + 3)
            rope_into(k_ps, kT[:, nsl0], nsl0)
            v_stage(v_ps, 0)
            rope_into(q0_ps, qT[:, 0, nsl0], nsl0)

            # chunk-0 pending work: q1..q3 projections streamed through
            # attention(0)'s drains. The chunk-1 h prefetch is emitted right
            # after the LAST q3 group reading each chunk-0 sub-tile (the
            # hpool buffer reuse must come after every reader is emitted),
            # and the wo column blocks trail at the end.
            pending = []
            q_state = {}
            hts_next = []

            def make_q_group(t, pr, hts, nsl):
                def emit():
                    if t not in q_state:
                        q_state[t] = {
                            "ps": pproj.tile([128, 512], F32, tag="proj",
                                             name=f"q_ps_0_{t}"),
                            "ops": _dr_ops(w_ap_of(wq_sb, t), h_ap_of(hts),
                                           ND),
                        }
                    st = q_state[t]
                    emit_chain(st["ps"], st["ops"], 3 * pr, 3 * pr + 3)
                    if pr == ND // 2 - 1:
                        rope_into(st["ps"], qT[:, t, nsl], nsl)
                        del q_state[t]
                return emit

            def make_h1_prefetch(dq):
                def emit():
                    t_h = hpool.tile([128, 4, 2, 512], F8, tag="ht",
                                     name=f"ht_1_{dq}")
                    assert len(hts_next) == dq
                    hts_next.append(t_h)
                    ht_dma(hts_next, 1, dq)
                return emit

            def make_wo_dma(cb):
                def emit():
                    csl = slice(cb * 1024, (cb + 1) * 1024)
                    nc.sync.dma_start(out=wo_sb[:, :, :, csl],
                                      in_=wo4[:, :, :, csl])
                return emit

            # per-head buckets: bucket t is fully emitted during head t's
            # drains, so head t+1's scores (which need qT[t+1]) are never
            # enqueued ahead of the projection work they depend on.
            pending = [[], [], [], []]
            for t in range(1, QH):
                for pr in range(ND // 2):
                    pending[t - 1].append(make_q_group(t, pr, hts0, nsl0))
                    if t == QH - 1 and pr % 2 == 1:
                        pending[t - 1].append(make_h1_prefetch(pr // 2))
            for cb in range(4):
                pending[3].append(make_wo_dma(cb))

            hts_cur = hts_next
            for I in range(NI):
                nsl = slice(I * 512, (I + 1) * 512)
                hts = hts_cur

                if I > 0:
                    # ---- projections (h prefetched; PE-bound) ------------
                    k_ps = pproj.tile([128, 512], F32, tag="proj")
                    emit_chain(k_ps, _dr_ops(w_ap_of(wk_sb), h_ap_of(hts),
                                             ND), 0, 3 * (ND // 2))
                    rope_into(k_ps, kT[:, nsl], nsl)

                    v_ps = pproj.tile([128, 512], F32, tag="proj")
                    emit_chain(v_ps, _dr_ops(w_ap_of(wv_sb), h_ap_of(hts),
                                             ND), 0, 3 * (ND // 2))
                    v_stage(v_ps, I)

                    for t in range(QH):
                        q_ps = pproj.tile([128, 512], F32, tag="proj")
                        emit_chain(q_ps, _dr_ops(w_ap_of(wq_sb, t),
                                                 h_ap_of(hts), ND),
                                   0, 3 * (ND // 2))
                        rope_into(q_ps, qT[:, t, nsl], nsl)

                    # prefetch next pass's h slice
                    if I + 1 < NI:
                        hts_cur = []
                        for dq in range(8):
                            t_h = hpool.tile([128, 4, 2, 512], F8, tag="ht",
                                             name=f"ht_{I + 1}_{dq}")
                            hts_cur.append(t_h)
                            ht_dma(hts_cur, I + 1, dq)

                # ---- attention for q-chunk I, with pending PE work
                # interleaved into the drain loop ----------------------
                njt = 4 * I + 4   # k-tiles in causal range of this chunk
                state = {}

                def start_head(t, I=I):
                    o_ps = [
                        popsum.tile([128, 132], F32, tag="opsum",
                                    name=f"o_ps_{I}_{t}_{il}")
                        for il in range(4)
                    ]
                    return {"o_ps": o_ps}

                def finalize_il(t, il, I=I):
                    o_ps = state[t]["o_ps"]
                    i_abs = 4 * I + il
                    recip = small.tile([128, 1], F32, tag="recip")
                    nc.vector.reciprocal(recip, o_ps[il][:, 128:129])
                    # osc carries the fp8 pre-scale SA so the casts are
                    # plain copy/sub ops that can run on the Pool engine
                    osc = small.tile([128, 128], F16, tag="osc")
                    nc.vector.tensor_scalar(
                        osc, o_ps[il][:, 0:128], recip, SA,
                        mybir.AluOpType.mult, mybir.AluOpType.mult,
                    )
                    tps = popsum.tile([128, 132], F16, tag="opsum",
                                      name=f"tps_{I}_{t}_{il}")
                    nc.tensor.transpose(tps[:, 0:128], osc, ident16)
                    att16 = small.tile([128, 128], F16, tag="att16")
                    nc.vector.tensor_copy(att16, tps[:, 0:128])
                    tok = slice(i_abs * 128, (i_abs + 1) * 128)
                    nc.gpsimd.tensor_copy(attn4[:, t, 0, tok], att16)
                    nc.gpsimd.tensor_sub(
                        attn4[:, t, 1, tok], att16, attn4[:, t, 0, tok]
                    )

                def drain(t, jprev, s_ps, I=I):
                    m = jprev - 4 * I
                    q_off = 128 * m if m > 0 else 0
                    pt = ptpool.tile([128, 512], BF16, tag="pt")
                    nc.scalar.activation(
                        pt[:, q_off:512], s_ps[:, q_off:512],
                        mybir.ActivationFunctionType.Exp,
                        scale=SCALE,
                    )
                    o_ps = state[t]["o_ps"]
                    for il in range(4):
                        i_abs = 4 * I + il
                        if jprev <= i_abs:
                            nc.tensor.matmul(
                                o_ps[il][:, 0:129],
                                pt[:, il * 128:(il + 1) * 128],
                                vA[:, jprev, 0:129],
                                start=(jprev == 0),
                                stop=(jprev == i_abs),
                            )
                            if jprev == i_abs:
                                finalize_il(t, il)

                prev = None
                for t in range(QH):
                    state[t] = start_head(t)
                    bucket = pending[t]
                    bucket_total = len(bucket)
                    bucket_done = 0
                    for j in range(njt):
                        mj = j - 4 * I
                        q_off = 128 * mj if mj > 0 else 0
                        s_ps = pwork.tile([128, 512], F32, tag="work")
                        diag = mj >= 0
                        nc.tensor.matmul(
                            s_ps[:, q_off:512],
                            kT[:, j * 128:(j + 1) * 128],
                            qT[:, t, I * 512 + q_off:(I + 1) * 512],
                            start=True, stop=not diag,
                        )
                        if diag:
                            # -60000 causal bias onto the diagonal block
                            nc.tensor.matmul(
                                s_ps[:, q_off:q_off + 128],
                                ident16, maskb,
                                start=False, stop=True,
                                skip_group_check=True,
                            )
                        if prev is not None:
                            tp_, jp_, sp_ = prev
                            drain(tp_, jp_, sp_)
                            target = (j + 1) * bucket_total // njt
                            while bucket and bucket_done < target:
                                bucket.pop(0)()
                                bucket_done += 1
                        elif bucket:
                            # prime the PE while the first exp runs
                            bucket.pop(0)()
                            bucket_done += 1
                        prev = (t, j, s_ps)
                    for fn in bucket:
                        fn()
                tp_, jp_, sp_ = prev
                drain(tp_, jp_, sp_)
                last = I == NI - 1
                units = [make_oproj_unit(I, il, qtr, split_dma=last,
                                          act_evict=last or I <= 1,
                                          use_pwork=last and (il + qtr) % 2 == 1,
                                          fine=last and il == 3 and qtr == 3)
                         for il in range(4) for qtr in range(4)]
                pending = [units[0:4], units[4:8], units[8:12], units[12:16]]

            for bucket in pending:
                for fn in bucket:
                    fn()
    nc.finalize()
    return nc


def _hilo(x, scale):
    """Split x*scale into fp8e4 (hi, lo) pieces."""
    import ml_dtypes

    xs = (x * scale).astype(np.float32)
    hi = xs.astype(ml_dtypes.float8_e4m3)
    lo = (xs - hi.astype(np.float32)).astype(ml_dtypes.float8_e4m3)
    return hi, lo


def _pack4(xT, scale, nd, ncols, lhs):
    """Pack xT [nd*128, ncols] into [128, nd, 2, ncols] fp8 pieces.
    lhs=True -> pieces (hi, lo); False -> (lo, hi)."""
    import ml_dtypes

    hi, lo = _hilo(xT, scale)
    outp = np.empty((128, nd, 2, ncols), dtype=ml_dtypes.float8_e4m3)
    h3 = hi.reshape(nd, 128, ncols)
    l3 = lo.reshape(nd, 128, ncols)
    if lhs:
        outp[:, :, 0, :] = h3.transpose(1, 0, 2)
        outp[:, :, 1, :] = l3.transpose(1, 0, 2)
    else:
        outp[:, :, 0, :] = l3.transpose(1, 0, 2)
        outp[:, :, 1, :] = h3.transpose(1, 0, 2)
    return outp


def _pack_inputs(h, position_ids, wq, wk, wv, wo):
    """Host-side shard + transpose + fp8 hi/lo split. Per-core input maps."""
    import ml_dtypes

    h4 = _pack4(np.ascontiguousarray(h.T), SH, ND, S, lhs=False)

    # RoPE tables in transposed orientation, halves duplicated / sign-folded.
    inv = 1.0 / (1e6 ** (np.arange(0, HD, 2, dtype=np.float64) / HD))
    fr = position_ids.astype(np.float64)[None, :] * inv[:, None]   # [64, S]
    cosT = np.cos(fr).astype(np.float16)
    sinT = np.sin(fr).astype(np.float16)
    cosd = np.concatenate([cosT, cosT], axis=0)                    # [128, S]
    sind = np.concatenate([-sinT, sinT], axis=0)
    iden16 = np.eye(128, dtype=np.float16)
    idenbf = np.eye(128).astype(ml_dtypes.bfloat16)
    p_i = np.arange(128)[:, None]
    f_i = np.arange(128)[None, :]
    maskd = np.where(f_i - p_i >= 0, 0.0, MASK_NEG).astype(np.float16)

    in_maps = []
    for c in range(NCORES):
        wq_c = wq[c * 512:(c + 1) * 512, :]          # [512, HID]
        wk_c = wk[c * 128:(c + 1) * 128, :]
        wv_c = wv[c * 128:(c + 1) * 128, :]
        wo_c = wo[:, c * 512:(c + 1) * 512]          # [HID, 512]
        woT = np.ascontiguousarray(wo_c.T)           # [512, HID]
        wqT = np.ascontiguousarray(wq_c.T)           # [HID, 512]
        wq4 = np.stack(
            [_pack4(np.ascontiguousarray(wqT[:, t * 128:(t + 1) * 128]),
                    SW, ND, 128, lhs=True) for t in range(QH)],
            axis=1,
        )                                            # [128, QH, ND, 2, 128]
        in_maps.append({
            "h4": h4,
            "wq4": wq4,
            "wk4": _pack4(np.ascontiguousarray(wk_c.T), SW, ND, 128, lhs=True),
            "wv4": _pack4(np.ascontiguousarray(wv_c.T), SW, ND, 128, lhs=True),
            "wo4": _pack4(woT, SW, QH, HID, lhs=False),
            "cosd": cosd,
            "sind": sind,
            "identd": iden16,
            "identb": idenbf,
            "maskd": maskd,
        })
    return in_maps


def kernel(h, position_ids, wq, wk, wv, wo):
    global LAST_RESULTS
    from concourse.bass_utils import run_bass_kernel_spmd

    if "nc" not in _CACHE:
        _CACHE["nc"] = _build_program()
    nc = _CACHE["nc"]

    in_maps = _pack_inputs(
        np.asarray(h, dtype=np.float32),
        np.asarray(position_ids),
        np.asarray(wq, dtype=np.float32),
        np.asarray(wk, dtype=np.float32),
        np.asarray(wv, dtype=np.float32),
        np.asarray(wo, dtype=np.float32),
    )

    trace = bool(int(os.environ.get("KERNEL_TRACE", "0")))
    res = run_bass_kernel_spmd(
        nc, in_maps, core_ids=list(range(NCORES)), trace=trace
    )
    LAST_RESULTS = res

    acc = np.zeros((S, HID), dtype=np.float32)
    for r in res.results:
        acc += r["out"].astype(np.float32)
    return acc


# revision 30
# speedup vs baseline: 1.0372x; 1.0014x over previous
"""Mixtral GQA attention block (B=1, S=2048, HID=4096, NH=32, NKV=8, HD=128),
8-way tensor-parallel over heads on trn2: each core owns 4 query heads + 1 KV
head (one GQA group), computes its partial output projection, host sums the
8 partials.

Device layout notes:
  - All matmul operands are staged transposed (contraction dim on partitions).
    Host pre-packs every tensor partition-major so DMAs are identity copies.
  - The Q/K/V and output projections run as fp8e4 DoubleRow matmuls with a
    3-term hi/lo error compensation: x@w ~= xh@wh + xh@wl + xl@wh. Pieces are
    packed [128, ktile, piece, N] with lhsT pieces (hi, lo) and rhs pieces
    (lo, hi); per ktile pair one DR instr covers both hi@hi products and one
    DR instr per ktile covers both cross products -> 0.75x the f16 PE cost
    at ~1e-3 accuracy. Tensors are pre-scaled into fp8's normal range
    (h x32, weights x1024, attn x16) and the product descale is folded into
    the PSUM eviction ops.
  - Scores are computed transposed (S^T[k,q]) in f16 so the exp'd tiles
    directly serve as lhsT for the P@V matmul; softmax denominator comes from
    an appended ones-column on V; no max-subtraction. Causal masking of the
    diagonal-band blocks is a -60000 bias added INTO the scores PSUM group
    via an identity-lhsT matmul, so exp needs no follow-up mask multiply.
  - RoPE is applied in the transposed orientation; the half-swap crosses
    partitions and runs as two partition-offset SBUF->SBUF DMAs.
  - The attention drain loop is Activation-bound (exp), so independent PE
    work is software-pipelined into it through a pending-work queue:
    chunk 0 streams the q1..q3 projections through attention(0); chunks
    1..3 stream the previous chunk's output projection. Chunk 0's k/v/q0
    projections are 3-way interleaved per ktile pair so the PE tracks the
    cold-start h DMA stream; DMAs are ordered by first use.
"""

import math
import os
import sys

import numpy as np

sys.path.insert(0, "/opt/trn_rl_repo")

import concourse.bass as bass
import concourse.tile as tile
from concourse import bacc
from concourse import mybir

S = 2048
HID = 4096
NH, NKV, HD = 32, 8, 128
NCORES = 8
QH = NH // NCORES      # 4 query heads per core
ND = HID // 128        # 32 contraction chunks
NI = S // 512          # 4 q-chunks of 512
NJ = S // 128          # 16 k-tiles of 128
SCALE = 1.0 / math.sqrt(HD)

F16 = mybir.dt.float16
BF16 = mybir.dt.bfloat16
F32 = mybir.dt.float32
F8 = mybir.dt.float8e4
DR = mybir.MatmulPerfMode.DoubleRow

SH = 32.0        # fp8 pre-scale on h
SW = 1024.0      # fp8 pre-scale on wq/wk/wv/wo
SA = 16.0        # fp8 pre-scale on attention output
DESCALE_QKV = float(1.0 / (SH * SW))
DESCALE_O = float(1.0 / (SA * SW))
MASK_NEG = -60000.0

_CACHE = {}
LAST_RESULTS = None


def _dr_ops(w_ap, h_ap, nd):
    """(lhsT, rhs) DR operand pairs for the 3-term hi/lo product over `nd`
    ktiles, in ktile-streaming order. w pieces are (hi, lo), h (lo, hi)."""
    ops = []
    for a in range(0, nd, 2):
        ops.append((w_ap(a, 0, False), h_ap(a, 1, False)))      # hi@hi pair
        ops.append((w_ap(a, None, True), h_ap(a, None, True)))  # cross a
        ops.append((w_ap(a + 1, None, True), h_ap(a + 1, None, True)))
    return ops


def _build_program():
    nc = bacc.Bacc()

    h4 = nc.declare_dram_parameter("h4", [128, ND, 2, S], F8, isOutput=False)
    wq4 = nc.declare_dram_parameter("wq4", [128, QH, ND, 2, 128], F8,
                                    isOutput=False)
    wk4 = nc.declare_dram_parameter("wk4", [128, ND, 2, 128], F8, isOutput=False)
    wv4 = nc.declare_dram_parameter("wv4", [128, ND, 2, 128], F8, isOutput=False)
    wo4 = nc.declare_dram_parameter("wo4", [128, QH, 2, HID], F8, isOutput=False)
    cosd = nc.declare_dram_parameter("cosd", [128, S], F16, isOutput=False)
    identd = nc.declare_dram_parameter("identd", [128, 128], F16, isOutput=False)
    identb = nc.declare_dram_parameter("identb", [128, 128], BF16, isOutput=False)
    maskd = nc.declare_dram_parameter("maskd", [128, 128], F16, isOutput=False)
    sind = nc.declare_dram_parameter("sind", [128, S], F16, isOutput=False)
    out = nc.declare_dram_parameter("out", [S, HID], F16, isOutput=True)

    with tile.TileContext(nc) as tc:
        with (
            tc.tile_pool(name="consts", bufs=1) as consts,
            tc.tile_pool(name="hpool", bufs=10) as hpool,
            tc.tile_pool(name="ptpool", bufs=6) as ptpool,
            tc.tile_pool(name="rtmp", bufs=3) as rtmp,
            tc.tile_pool(name="small", bufs=8) as small,
            tc.tile_pool(name="orow", bufs=5) as orowp,
            tc.tile_pool(name="pproj", bufs=2, space="PSUM") as pproj,
            tc.tile_pool(name="pwork", bufs=2, space="PSUM") as pwork,
            tc.tile_pool(name="popsum", bufs=4, space="PSUM") as popsum,
        ):
            wq_sb = consts.tile([128, QH, ND, 2, 128], F8)
            cos_sb = consts.tile([128, S], F16)
            sin_sb = consts.tile([128, S], F16)
            wk_sb = consts.tile([128, ND, 2, 128], F8)
            wv_sb = consts.tile([128, ND, 2, 128], F8)
            wo_sb = consts.tile([128, QH, 2, HID], F8)
            ident16 = consts.tile([128, 128], F16)
            identbf = consts.tile([128, 128], BF16)
            maskb = consts.tile([128, 128], F16)

            hts0 = []
            for dq in range(8):
                t_h = hpool.tile([128, 4, 2, 512], F8, tag="ht",
                                 name=f"ht_0_{dq}")
                hts0.append(t_h)

            def ht_dma(tiles, I, dq):
                nc.sync.dma_start(
                    out=tiles[dq],
                    in_=h4[:, dq * 4:(dq + 1) * 4, :, I * 512:(I + 1) * 512],
                )

            def wq_dma(t, half):
                dsl = slice(half * 16, (half + 1) * 16)
                nc.sync.dma_start(out=wq_sb[:, t, dsl, :, :],
                                  in_=wq4[:, t, dsl, :, :])

            # Startup DMA order = first-use order: k/v/q0 weights lead the
            # cold h stream; tables next; q1..q3 heads land during
            # attention(0); wo column blocks land before attention(1).
            nc.sync.dma_start(out=wk_sb[:, 0:8, :, :], in_=wk4[:, 0:8, :, :])
            nc.sync.dma_start(out=hts0[0][:, 0:2, :, :],
                              in_=h4[:, 0:2, :, 0:512])
            nc.sync.dma_start(out=wv_sb[:, 0:8, :, :], in_=wv4[:, 0:8, :, :])
            nc.sync.dma_start(out=hts0[0][:, 2:4, :, :],
                              in_=h4[:, 2:4, :, 0:512])
            nc.sync.dma_start(out=wq_sb[:, 0, 0:8, :, :],
                              in_=wq4[:, 0, 0:8, :, :])
            nc.sync.dma_start(out=wk_sb[:, 8:16, :, :], in_=wk4[:, 8:16, :, :])
            nc.sync.dma_start(out=wv_sb[:, 8:16, :, :], in_=wv4[:, 8:16, :, :])
            nc.sync.dma_start(out=wq_sb[:, 0, 8:16, :, :],
                              in_=wq4[:, 0, 8:16, :, :])
            nc.sync.dma_start(out=wk_sb[:, 16:32, :, :], in_=wk4[:, 16:32, :, :])
            ht_dma(hts0, 0, 1)
            nc.sync.dma_start(out=wv_sb[:, 16:32, :, :], in_=wv4[:, 16:32, :, :])
            ht_dma(hts0, 0, 2)
            wq_dma(0, 1)
            ht_dma(hts0, 0, 3)
            for dq in range(4, 8):
                ht_dma(hts0, 0, dq)
            nc.sync.dma_start(out=cos_sb[:, 0:512], in_=cosd[:, 0:512])
            nc.sync.dma_start(out=sin_sb[:, 0:512], in_=sind[:, 0:512])
            nc.sync.dma_start(out=identbf, in_=identb[:, :])
            nc.sync.dma_start(out=maskb, in_=maskd[:, :])
            nc.sync.dma_start(out=ident16, in_=identd[:, :])
            wq_dma(1, 0)
            wq_dma(1, 1)
            nc.sync.dma_start(out=cos_sb[:, 512:S], in_=cosd[:, 512:S])
            nc.sync.dma_start(out=sin_sb[:, 512:S], in_=sind[:, 512:S])
            for t in range(2, QH):
                wq_dma(t, 0)
                wq_dma(t, 1)

            qT = consts.tile([128, QH, S], F16)    # roped q, transposed
            kT = consts.tile([128, S], F16)        # roped k, transposed
            # V' tiles: per k-tile j, [128 tokens, 128 ch + ones column]
            vA = consts.tile([128, NJ, 132], BF16)
            for j in range(NJ):
                nc.vector.memset(vA[:, j, 128:129], 1.0)
            # attn output pieces for the fp8 O-projection: [hd, head, piece, S]
            attn4 = consts.tile([128, QH, 2, S], F8)

            def w_ap_of(w_tile, t=None):
                if t is None:
                    def w_ap(a, piece, single):
                        if single:
                            return w_tile[:, a, 0:2, :]
                        return w_tile[:, a:a + 2, piece, :]
                else:
                    def w_ap(a, piece, single):
                        if single:
                            return w_tile[:, t, a, 0:2, :]
                        return w_tile[:, t, a:a + 2, piece, :]
                return w_ap

            def h_ap_of(hts):
                def h_ap(a, piece, single):
                    if single:
                        return hts[a // 4][:, a % 4, 0:2, :]
                    return hts[a // 4][:, a % 4:a % 4 + 2, piece, :]
                return h_ap

            def emit_chain(ps, ops, lo, hi_i):
                for i in range(lo, hi_i):
                    nc.tensor.matmul(
                        ps, ops[i][0], ops[i][1],
                        start=(i == 0), stop=(i == len(ops) - 1),
                        perf_mode=DR,
                    )

            def rope_into(ps, dst, nsl):
                """ps: PSUM [128, 512] f32 pre-rope (transposed layout,
                scaled by SH*SW). dst: f16 SBUF slice. The half-swap
                crosses partitions -> two partition-offset DMAs."""
                cpy = rtmp.tile([128, 512], F16, tag="ropecpy")
                nc.scalar.mul(cpy, ps, DESCALE_QKV)
                sw = rtmp.tile([128, 512], F16, tag="ropesw")
                nc.sync.dma_start(out=sw[0:64, :], in_=cpy[64:128, :])
                nc.sync.dma_start(out=sw[64:128, :], in_=cpy[0:64, :])
                tmp2 = rtmp.tile([128, 512], F16, tag="ropecos")
                nc.gpsimd.tensor_mul(tmp2, cpy, cos_sb[:, nsl])
                nc.gpsimd.tensor_mul(sw, sw, sin_sb[:, nsl])
                nc.vector.tensor_add(dst, tmp2, sw)

            def v_stage(v_ps, I):
                vt_sb = small.tile([128, 512], BF16, tag="vt")
                nc.vector.tensor_scalar_mul(vt_sb, v_ps, DESCALE_QKV)
                for jj in range(4):
                    tps = pwork.tile([128, 128], BF16, tag="work")
                    nc.tensor.transpose(
                        tps, vt_sb[:, jj * 128:(jj + 1) * 128], identbf
                    )
                    nc.vector.tensor_copy(vA[:, 4 * I + jj, 0:128], tps)

            # ---- O-projection emission units ------------------------------
            def make_oproj_unit(I, il, qtr, split_dma=False, act_evict=False,
                                use_pwork=False, fine=False):
                i_abs = 4 * I + il
                tok = slice(i_abs * 128, (i_abs + 1) * 128)

                def a_ap(a, piece, single):
                    if single:
                        return attn4[:, a, 0:2, tok]
                    return attn4[:, a:a + 2, piece, tok]

                def emit():
                    orow = orowp.tile([128, 1024], F16, tag="orow")
                    for mc in range(2):
                        mq = qtr * 1024 + mc * 512

                        def w_ap(a, piece, single, mq=mq):
                            if single:
                                return wo_sb[:, a, 0:2, mq:mq + 512]
                            return wo_sb[:, a:a + 2, piece, mq:mq + 512]

                        pool = pwork if use_pwork else pproj
                        ptag = "work" if use_pwork else "proj"
                        op_ps = pool.tile([128, 512], F32, tag=ptag)
                        ops = _dr_ops(a_ap, w_ap, QH)
                        emit_chain(op_ps, ops, 0, len(ops))
                        osl = orow[:, mc * 512:(mc + 1) * 512]
                        if fine and mc == 1:
                            for qq in range(2):
                                qsl = slice(qq * 256, (qq + 1) * 256)
                                eng = nc.scalar.mul if qq else \
                                    nc.vector.tensor_scalar_mul
                                eng(osl[:, qsl], op_ps[:, qsl], DESCALE_O)
                                nc.sync.dma_start(
                                    out=out[i_abs * 128:(i_abs + 1) * 128,
                                            mq + qq * 256:mq + qq * 256 + 256],
                                    in_=osl[:, qsl],
                                )
                            continue
                        if act_evict and mc == 1:
                            nc.scalar.mul(osl, op_ps, DESCALE_O)
                        else:
                            nc.vector.tensor_scalar_mul(osl, op_ps, DESCALE_O)
                        if split_dma:
                            nc.sync.dma_start(
                                out=out[i_abs * 128:(i_abs + 1) * 128,
                                        mq:mq + 512],
                                in_=osl,
                            )
                    if not split_dma:
                        nc.sync.dma_start(
                            out=out[i_abs * 128:(i_abs + 1) * 128,
                                    qtr * 1024:(qtr + 1) * 1024],
                            in_=orow,
                        )
                return emit

            # ---- chunk 0 cold start: 3-way interleaved k/v/q0 -------------
            nsl0 = slice(0, 512)
            hap0 = h_ap_of(hts0)
            k_ps = pproj.tile([128, 512], F32, tag="proj")
            v_ps = pproj.tile([128, 512], F32, tag="proj")
            q0_ps = popsum.tile([128, 512], F32, tag="opsum", name="q_ps_0_0")
            k_ops = _dr_ops(w_ap_of(wk_sb), hap0, ND)
            v_ops = _dr_ops(w_ap_of(wv_sb), hap0, ND)
            q0_ops = _dr_ops(w_ap_of(wq_sb, 0), hap0, ND)
            for pr in range(ND // 2):
                emit_chain(k_ps, k_ops, 3 * pr, 3 * pr + 3)
                emit_chain(v_ps, v_ops, 3 * pr, 3 * pr + 3)
                emit_chain(q0_ps, q0_ops, 3 * pr, 3 * pr + 3)
            rope_into(k_ps, kT[:, nsl0], nsl0)
            v_stage(v_ps, 0)
            rope_into(q0_ps, qT[:, 0, nsl0], nsl0)

            # chunk-0 pending work: q1..q3 projections streamed through
            # attention(0)'s drains. The chunk-1 h prefetch is emitted right
            # after the LAST q3 group reading each chunk-0 sub-tile (the
            # hpool buffer reuse must come after every reader is emitted),
            # and the wo column blocks trail at the end.
            pending = []
            q_state = {}
            hts_next = []

            def make_q_group(t, pr, hts, nsl):
                def emit():
                    if t not in q_state:
                        q_state[t] = {
                            "ps": pproj.tile([128, 512], F32, tag="proj",
                                             name=f"q_ps_0_{t}"),
                            "ops": _dr_ops(w_ap_of(wq_sb, t), h_ap_of(hts),
                                           ND),
                        }
                    st = q_state[t]
                    emit_chain(st["ps"], st["ops"], 3 * pr, 3 * pr + 3)
                    if pr == ND // 2 - 1:
                        rope_into(st["ps"], qT[:, t, nsl], nsl)
                        del q_state[t]
                return emit

            def make_h1_prefetch(dq):
                def emit():
                    t_h = hpool.tile([128, 4, 2, 512], F8, tag="ht",
                                     name=f"ht_1_{dq}")
                    assert len(hts_next) == dq
                    hts_next.append(t_h)
                    ht_dma(hts_next, 1, dq)
                return emit

            def make_wo_dma(cb):
                def emit():
                    csl = slice(cb * 1024, (cb + 1) * 1024)
                    nc.sync.dma_start(out=wo_sb[:, :, :, csl],
                                      in_=wo4[:, :, :, csl])
                return emit

            # per-head buckets: bucket t is fully emitted during head t's
            # drains, so head t+1's scores (which need qT[t+1]) are never
            # enqueued ahead of the projection work they depend on.
            pending = [[], [], [], []]
            for t in range(1, QH):
                for pr in range(ND // 2):
                    pending[t - 1].append(make_q_group(t, pr, hts0, nsl0))
                    if t == QH - 1 and pr % 2 == 1:
                        pending[t - 1].append(make_h1_prefetch(pr // 2))
            for cb in range(4):
                pending[3].append(make_wo_dma(cb))

            hts_cur = hts_next
            for I in range(NI):
                nsl = slice(I * 512, (I + 1) * 512)
                hts = hts_cur

                if I > 0:
                    # ---- projections (h prefetched; PE-bound) ------------
                    k_ps = pproj.tile([128, 512], F32, tag="proj")
                    emit_chain(k_ps, _dr_ops(w_ap_of(wk_sb), h_ap_of(hts),
                                             ND), 0, 3 * (ND // 2))
                    rope_into(k_ps, kT[:, nsl], nsl)

                    v_ps = pproj.tile([128, 512], F32, tag="proj")
                    emit_chain(v_ps, _dr_ops(w_ap_of(wv_sb), h_ap_of(hts),
                                             ND), 0, 3 * (ND // 2))
                    v_stage(v_ps, I)

                    for t in range(QH):
                        q_ps = pproj.tile([128, 512], F32, tag="proj")
                        emit_chain(q_ps, _dr_ops(w_ap_of(wq_sb, t),
                                                 h_ap_of(hts), ND),
                                   0, 3 * (ND // 2))
                        rope_into(q_ps, qT[:, t, nsl], nsl)

                    # prefetch next pass's h slice
                    if I + 1 < NI:
                        hts_cur = []
                        for dq in range(8):
                            t_h = hpool.tile([128, 4, 2, 512], F8, tag="ht",
                                             name=f"ht_{I + 1}_{dq}")
                            hts_cur.append(t_h)
                            ht_dma(hts_cur, I + 1, dq)

                # ---- attention for q-chunk I, with pending PE work
                # interleaved into the drain loop ----------------------
                njt = 4 * I + 4   # k-tiles in causal range of this chunk
                state = {}

                def start_head(t, I=I):
                    o_ps = [
                        popsum.tile([128, 132], F32, tag="opsum",
                                    name=f"o_ps_{I}_{t}_{il}")
                        for il in range(4)
                    ]
                    return {"o_ps": o_ps}

                def finalize_il(t, il, I=I):
                    o_ps = state[t]["o_ps"]
                    i_abs = 4 * I + il
                    recip = small.tile([128, 1], F32, tag="recip")
                    nc.vector.reciprocal(recip, o_ps[il][:, 128:129])
                    # osc carries the fp8 pre-scale SA so the casts are
                    # plain copy/sub ops that can run on the Pool engine
                    osc = small.tile([128, 128], F16, tag="osc")
                    nc.vector.tensor_scalar(
                        osc, o_ps[il][:, 0:128], recip, SA,
                        mybir.AluOpType.mult, mybir.AluOpType.mult,
                    )
                    tps = popsum.tile([128, 132], F16, tag="opsum",
                                      name=f"tps_{I}_{t}_{il}")
                    nc.tensor.transpose(tps[:, 0:128], osc, ident16)
                    att16 = small.tile([128, 128], F16, tag="att16")
                    nc.vector.tensor_copy(att16, tps[:, 0:128])
                    tok = slice(i_abs * 128, (i_abs + 1) * 128)
                    nc.gpsimd.tensor_copy(attn4[:, t, 0, tok], att16)
                    nc.gpsimd.tensor_sub(
                        attn4[:, t, 1, tok], att16, attn4[:, t, 0, tok]
                    )

                def drain(t, jprev, s_ps, I=I):
                    m = jprev - 4 * I
                    q_off = 128 * m if m > 0 else 0
                    pt = ptpool.tile([128, 512], BF16, tag="pt")
                    nc.scalar.activation(
                        pt[:, q_off:512], s_ps[:, q_off:512],
                        mybir.ActivationFunctionType.Exp,
                        scale=SCALE,
                    )
                    o_ps = state[t]["o_ps"]
                    for il in range(4):
                        i_abs = 4 * I + il
                        if jprev <= i_abs:
                            nc.tensor.matmul(
                                o_ps[il][:, 0:129],
                                pt[:, il * 128:(il + 1) * 128],
                                vA[:, jprev, 0:129],
                                start=(jprev == 0),
                                stop=(jprev == i_abs),
                            )
                            if jprev == i_abs:
                                finalize_il(t, il)

                prev = None
                for t in range(QH):
                    state[t] = start_head(t)
                    bucket = pending[t]
                    bucket_total = len(bucket)
                    bucket_done = 0
                    for j in range(njt):
                        mj = j - 4 * I
                        q_off = 128 * mj if mj > 0 else 0
                        s_ps = pwork.tile([128, 512], F32, tag="work")
                        diag = mj >= 0
                        nc.tensor.matmul(
                            s_ps[:, q_off:512],
                            kT[:, j * 128:(j + 1) * 128],
                            qT[:, t, I * 512 + q_off:(I + 1) * 512],
                            start=True, stop=not diag,
                        )
                        if diag:
                            # -60000 causal bias onto the diagonal block
                            nc.tensor.matmul(
                                s_ps[:, q_off:q_off + 128],
                                ident16, maskb,
                                start=False, stop=True,
                                skip_group_check=True,
                            )
                        if prev is not None:
                            tp_, jp_, sp_ = prev
                            drain(tp_, jp_, sp_)
                            target = (j + 1) * bucket_total // njt
                            while bucket and bucket_done < target:
                                bucket.pop(0)()
                                bucket_done += 1
                        elif bucket:
                            # prime the PE while the first exp runs
                            bucket.pop(0)()
                            bucket_done += 1
                        prev = (t, j, s_ps)
                    for fn in bucket:
                        fn()
                tp_, jp_, sp_ = prev
                drain(tp_, jp_, sp_)
                last = I == NI - 1
                units = [make_oproj_unit(I, il, qtr, split_dma=last,
                                          act_evict=last or I <= 1,
                                          use_pwork=last and (il + qtr) % 2 == 1,
                                          fine=last and il == 3 and qtr == 3)
                         for il in range(4) for qtr in range(4)]
                pending = [units[0:4], units[4:8], units[8:12], units[12:16]]

            for bucket in pending:
                for fn in bucket:
                    fn()
    nc.finalize()
    return nc


def _hilo(x, scale):
    """Split x*scale into fp8e4 (hi, lo) pieces."""
    import ml_dtypes

    xs = (x * scale).astype(np.float32)
    hi = xs.astype(ml_dtypes.float8_e4m3)
    lo = (xs - hi.astype(np.float32)).astype(ml_dtypes.float8_e4m3)
    return hi, lo


def _pack4(xT, scale, nd, ncols, lhs):
    """Pack xT [nd*128, ncols] into [128, nd, 2, ncols] fp8 pieces.
    lhs=True -> pieces (hi, lo); False -> (lo, hi)."""
    import ml_dtypes

    hi, lo = _hilo(xT, scale)
    outp = np.empty((128, nd, 2, ncols), dtype=ml_dtypes.float8_e4m3)
    h3 = hi.reshape(nd, 128, ncols)
    l3 = lo.reshape(nd, 128, ncols)
    if lhs:
        outp[:, :, 0, :] = h3.transpose(1, 0, 2)
        outp[:, :, 1, :] = l3.transpose(1, 0, 2)
    else:
        outp[:, :, 0, :] = l3.transpose(1, 0, 2)
        outp[:, :, 1, :] = h3.transpose(1, 0, 2)
    return outp


def _pack_inputs(h, position_ids, wq, wk, wv, wo):
    """Host-side shard + transpose + fp8 hi/lo split. Per-core input maps."""
    import ml_dtypes

    h4 = _pack4(np.ascontiguousarray(h.T), SH, ND, S, lhs=False)

    # RoPE tables in transposed orientation, halves duplicated / sign-folded.
    inv = 1.0 / (1e6 ** (np.arange(0, HD, 2, dtype=np.float64) / HD))
    fr = position_ids.astype(np.float64)[None, :] * inv[:, None]   # [64, S]
    cosT = np.cos(fr).astype(np.float16)
    sinT = np.sin(fr).astype(np.float16)
    cosd = np.concatenate([cosT, cosT], axis=0)                    # [128, S]
    sind = np.concatenate([-sinT, sinT], axis=0)
    iden16 = np.eye(128, dtype=np.float16)
    idenbf = np.eye(128).astype(ml_dtypes.bfloat16)
    p_i = np.arange(128)[:, None]
    f_i = np.arange(128)[None, :]
    maskd = np.where(f_i - p_i >= 0, 0.0, MASK_NEG).astype(np.float16)

    in_maps = []
    for c in range(NCORES):
        wq_c = wq[c * 512:(c + 1) * 512, :]          # [512, HID]
        wk_c = wk[c * 128:(c + 1) * 128, :]
        wv_c = wv[c * 128:(c + 1) * 128, :]
        wo_c = wo[:, c * 512:(c + 1) * 512]          # [HID, 512]
        woT = np.ascontiguousarray(wo_c.T)           # [512, HID]
        wqT = np.ascontiguousarray(wq_c.T)           # [HID, 512]
        wq4 = np.stack(
            [_pack4(np.ascontiguousarray(wqT[:, t * 128:(t + 1) * 128]),
                    SW, ND, 128, lhs=True) for t in range(QH)],
            axis=1,
        )                                            # [128, QH, ND, 2, 128]
        in_maps.append({
            "h4": h4,
            "wq4": wq4,
            "wk4": _pack4(np.ascontiguousarray(wk_c.T), SW, ND, 128, lhs=True),
            "wv4": _pack4(np.ascontiguousarray(wv_c.T), SW, ND, 128, lhs=True),
            "wo4": _pack4(woT, SW, QH, HID, lhs=False),
            "cosd": cosd,
            "sind": sind,
            "identd": iden16,
            "identb": idenbf,
            "maskd": maskd,
        })
    return in_maps


def kernel(h, position_ids, wq, wk, wv, wo):
    global LAST_RESULTS
    from concourse.bass_utils import run_bass_kernel_spmd

    if "nc" not in _CACHE:
        _CACHE["nc"] = _build_program()
    nc = _CACHE["nc"]

    in_maps = _pack_inputs(
        np.asarray(h, dtype=np.float32),
        np.asarray(position_ids),
        np.asarray(wq, dtype=np.float32),
        np.asarray(wk, dtype=np.float32),
        np.asarray(wv, dtype=np.float32),
        np.asarray(wo, dtype=np.float32),
    )

    trace = bool(int(os.environ.get("KERNEL_TRACE", "0")))
    res = run_bass_kernel_spmd(
        nc, in_maps, core_ids=list(range(NCORES)), trace=trace
    )
    LAST_RESULTS = res

    acc = np.zeros((S, HID), dtype=np.float32)
    for r in res.results:
        acc += r["out"].astype(np.float32)
    return acc
